# revision 1
# baseline (speedup 1.0000x reference)
"""Trainium2 Bass kernel for nn_DrugGCNncoder (2-layer GCN + max-pool + MLP).

Self-contained: accepts the FULL inputs of reference.setup_inputs(), shards
across 8 NeuronCores internally (dst-node/graph sharding; AllGather of the
layer-1 aggregate), returns the FULL [512, 128] output.
"""
import sys
for p in ("/opt/trn_rl_repo", "/root/.axon_site/_ro/trn_rl_repo"):
    if p not in sys.path:
        sys.path.insert(0, p)
import numpy as np
import concourse.bass as bass
import concourse.bacc as bacc
import concourse.mybir as mybir
from concourse import tile
from concourse.bass_utils import run_bass_kernel_spmd

import numpy as np

CHUNK = 32768
DSTW = 256          # window width in dst-node columns (S matrix free dim)
F1P = 128           # x padded feature count (512B rows)
F1 = 78
F2P = 320           # h1 padded feature count (1280B rows)
F2 = 300
FOUT = 128
G_PER_CORE = 64  # overridden by build_plan
N_CORES = 8
N_GRAPHS = 512


def _pack_idx16(idx, cap):
    """idx (valid list) -> [128, cap//16] int16, slot j at [j%16, j//16],
    padded with 0 (valid row 0), replicated 8x along partitions."""
    assert cap % 16 == 0 and len(idx) <= cap
    full = np.zeros(cap, np.int16)
    full[: len(idx)] = idx
    blk = full.reshape(cap // 16, 16).T  # [16, cap/16]
    return np.tile(blk, (8, 1))  # [128, cap/16]


def _windows_for_edges(dst_local, base_grid, limits=None):
    """Split dst-sorted edges into windows. Window w covers local dst range
    [base_grid[w], min(base_grid[w] + DSTW, limits[w])). Returns (lo, hi)
    edge index ranges per window."""
    out = []
    for i, b in enumerate(base_grid):
        top = b + DSTW if limits is None else min(b + DSTW, limits[i])
        lo = np.searchsorted(dst_local, b, side="left")
        hi = np.searchsorted(dst_local, top, side="left")
        out.append((lo, hi))
    return out


def build_plan(x, edge_index, batch, weights, t1_tiles=None, t2_tiles=None,
               n_graphs=512, n_cores=8):
    global G_PER_CORE, N_CORES, N_GRAPHS
    N_GRAPHS, N_CORES = n_graphs, n_cores
    G_PER_CORE = n_graphs // n_cores
    """Returns (cfg, per_core, shared) for kernel_build + runner."""
    N = x.shape[0]
    E = edge_index.shape[1]
    src = np.concatenate([edge_index[0], np.arange(N)]).astype(np.int64)
    dst = np.concatenate([edge_index[1], np.arange(N)]).astype(np.int64)
    deg = np.bincount(dst, minlength=N).astype(np.float64)
    dis = np.where(deg > 0, 1.0 / np.sqrt(deg), 0.0)
    norm = (dis[src] * dis[dst]).astype(np.float32)

    batch = batch.astype(np.int64)
    # graph -> node range (batch sorted). graphs may be empty.
    g_start = np.searchsorted(batch, np.arange(N_GRAPHS), side="left")
    g_end = np.searchsorted(batch, np.arange(N_GRAPHS), side="right")
    node_start = [int(g_start[c * G_PER_CORE]) for c in range(N_CORES)]
    node_start.append(N)
    nodes_per_core = [node_start[c + 1] - node_start[c] for c in range(N_CORES)]
    NMAX = ((max(nodes_per_core) + DSTW - 1) // DSTW) * DSTW
    NPAD_TOT = N_CORES * NMAX

    # global padded node id: node n in core c at local i -> c*NMAX + i
    core_of = np.searchsorted(np.asarray(node_start[1:]), np.arange(N), side="right")
    local_of = np.arange(N) - np.asarray(node_start)[core_of]
    pad_id = core_of * NMAX + local_of  # [N] int

    per_core_raw = []
    for c in range(N_CORES):
        sel = (dst >= node_start[c]) & (dst < node_start[c + 1])
        s, d, nm = src[sel], dst[sel], norm[sel]
        dl = d - node_start[c]
        order = np.argsort(dl, kind="stable")
        per_core_raw.append((s[order], dl[order], nm[order]))

    # ---- window construction -------------------------------------------
    def make_windows(core_edges, base_grid, src_ids, n_chunks, t_tiles,
                     limits=None):
        """Returns per-window dict lists. src_ids: per-edge source row id in
        the gather table (x: global node id; h1: padded global id)."""
        s_loc, dl, nm = core_edges
        wins = []
        for (lo, hi), b in zip(_windows_for_edges(dl, base_grid, limits),
                               base_grid):
            es, edl, enm = src_ids[lo:hi], dl[lo:hi] - b, nm[lo:hi]
            ch = es // CHUNK
            runs = []
            for k in range(n_chunks):
                m = ch == k
                runs.append((es[m] - k * CHUNK, edl[m], enm[m]))
            wins.append((b, runs))
        return wins

    # L1: fixed grid over local nodes
    shared_w = weights
    l1_cores, l2_cores = [], []
    n_chunks_x = (N + CHUNK - 1) // CHUNK
    n_chunks_h = (NPAD_TOT + CHUNK - 1) // CHUNK
    for c in range(N_CORES):
        s_loc, dl, nm = per_core_raw[c]
        grid1 = np.arange(0, NMAX, DSTW)
        l1_cores.append(make_windows((s_loc, dl, nm), grid1, s_loc, n_chunks_x, None))
        # L2: graph-aligned windows (clipped at graph end)
        base2, lim2 = [], []
        for g in range(c * G_PER_CORE, (c + 1) * G_PER_CORE):
            glo = g_start[g] - node_start[c]
            ghi = g_end[g] - node_start[c]
            for b in range(int(glo), int(ghi), DSTW):
                base2.append(b)
                lim2.append(int(ghi))
        base2 = np.asarray(base2, np.int64)
        l2_cores.append(
            make_windows((s_loc, dl, nm), base2, pad_id[s_loc], n_chunks_h,
                         None, limits=lim2)
        )

    # ---- capacity normalization across cores ---------------------------
    def normalize(cores_wins, n_chunks, t_tiles_fixed):
        """Pad window count to max across cores; compute per-(window,chunk)
        caps = max over cores, each rounded to 128; cap total <= t_tiles*128."""
        n_win = max(len(w) for w in cores_wins)
        for wlist in cores_wins:
            while len(wlist) < n_win:
                wlist.append((0, [(np.array([], np.int64),) * 3] * n_chunks))
        caps = np.zeros((n_win, n_chunks), np.int64)
        for wlist in cores_wins:
            for w, (b, runs) in enumerate(wlist):
                for k, (ri, rd, rn) in enumerate(runs):
                    caps[w, k] = max(caps[w, k], len(ri))
        caps = ((caps + 127) // 128) * 128
        t_tiles = int(caps.sum(axis=1).max()) // 128
        if t_tiles_fixed is not None:
            t_tiles = max(t_tiles, t_tiles_fixed)
        return n_win, caps, t_tiles

    n_win1, caps1, T1 = normalize(l1_cores, n_chunks_x, t1_tiles)
    n_win2, caps2, T2 = normalize(l2_cores, n_chunks_h, t2_tiles)

    # ---- emit per-core arrays ------------------------------------------
    def emit(cores_wins, caps, n_win, t_tiles, n_chunks):
        n_idx16 = int(caps.sum()) // 16  # total idx cols across windows/chunks
        out = []
        for wlist in cores_wins:
            idx16 = np.zeros((128, n_idx16), np.int16)
            dstl = np.full((n_win, 128, t_tiles), -1.0, np.float32)
            nrm = np.zeros((n_win, 128, t_tiles), np.float32)
            bases = np.zeros(n_win, np.int64)
            col16 = 0
            for w, (b, runs) in enumerate(wlist):
                bases[w] = b
                slot = 0
                for k in range(n_chunks):
                    cap = int(caps[w, k])
                    ri, rd, rn = runs[k]
                    idx16[:, col16 : col16 + cap // 16] = _pack_idx16(ri, cap)
                    n = len(ri)
                    sl = slot + np.arange(n)
                    dstl[w, sl % 128, sl // 128] = rd.astype(np.float32)
                    nrm[w, sl % 128, sl // 128] = rn
                    slot += cap
                    col16 += cap // 16
                assert slot <= t_tiles * 128
            out.append({"idx16": idx16, "dstl": dstl, "norm": nrm, "bases": bases})
        return out

    l1_data = emit(l1_cores, caps1, n_win1, T1, n_chunks_x)
    l2_data = emit(l2_cores, caps2, n_win2, T2, n_chunks_h)
    for d in l1_data:
        d["meta"] = np.concatenate([d["dstl"], d["norm"]], axis=2)
    for d in l2_data:
        d["meta"] = np.concatenate([d["dstl"], d["norm"]], axis=2)

    # per-window slot offsets (compile-time schedule constants)
    def sched(caps):
        # for each window: list of (chunk, cap, slot_off, idx16_off)
        rows = []
        col16 = 0
        for w in range(caps.shape[0]):
            slot = 0
            ent = []
            for k in range(caps.shape[1]):
                cap = int(caps[w, k])
                if cap > 0:
                    ent.append((k, cap, slot, col16))
                slot += cap
                col16 += cap // 16
            rows.append((ent, slot))
        return rows

    # ---- pooling masks --------------------------------------------------
    # window w (L2) belongs to graph (data per core). mask_neg[g_local] over
    # windows: 0 where window belongs to g, -3e38 elsewhere.
    n_win2_pad16 = ((n_win2 + 15) // 16) * 16
    pool_masks = []
    for c in range(N_CORES):
        m = np.full((G_PER_CORE, n_win2_pad16), np.float32(-3.0e38), np.float32)
        wlist = l2_cores[c]
        # recompute window->graph from bases
        glo = g_start[c * G_PER_CORE : (c + 1) * G_PER_CORE] - node_start[c]
        ghi = g_end[c * G_PER_CORE : (c + 1) * G_PER_CORE] - node_start[c]
        for w, (b, runs) in enumerate(wlist):
            # empty pad windows (b==0 with no edges): they reduce to garbage?
            # no: their h2T cols come from agg=0 -> relu(b2)=0; window 0 of a
            # core always belongs to graph 0's range start... pad windows get
            # base 0 which may alias graph 0; keep them masked out for all g.
            total = sum(len(r[0]) for r in runs)
            if total == 0 and w >= len([1 for (bb, rr) in wlist]):
                continue
            g = int(np.searchsorted(ghi, b, side="right"))
            # base b belongs to graph g if glo[g] <= b < ghi[g]
            if g < G_PER_CORE and glo[g] <= b < ghi[g]:
                m[g, w] = 0.0
        pool_masks.append(m)
    # NOTE: truly-empty pad windows have base 0; if graph 0 is non-empty and
    # has base-0 window, the pad window aliases it. Its pooled value is
    # relu(b2)>=0 from zero agg... To be safe, only the FIRST window with a
    # given (base,graph) pair keeps mask 0; duplicates masked.
    for c in range(N_CORES):
        seen = set()
        wlist = l2_cores[c]
        for w, (b, runs) in enumerate(wlist):
            total = sum(len(r[0]) for r in runs)
            key = int(b)
            if total == 0 and key in seen:
                pool_masks[c][:, w] = -3.0e38
            seen.add(key)

    # ---- packed weights (shared across cores) --------------------------
    W1, b1, W2, b2, W3, b3, W4, b4 = (
        weights["W1"], weights["b1"], weights["W2"], weights["b2"],
        weights["W3"], weights["b3"], weights["W4"], weights["b4"],
    )
    W1aug = np.zeros((80, 384), np.float32)
    W1aug[:F1, :F2] = W1
    W1aug[F1, :F2] = b1  # ones-row slot 78
    W2aug = np.zeros((304, F2P), np.float32)  # k rows 0..299 = W2, row 300 = b2
    W2aug[:F2, :F2] = W2
    W2aug[F2, :F2] = b2
    W3aug = np.zeros((304, 1024), np.float32)
    W3aug[:F2, :] = W3
    W3aug[F2, :] = b3
    W4aug = np.zeros((1024 + 128, FOUT), np.float32)  # extra 128-row chunk for bias
    W4aug[:1024, :] = W4
    W4aug[1024, :] = b4

    iota = np.tile(np.arange(DSTW, dtype=np.float32)[None, :], (128, 1))
    onesmat = np.zeros((128, DSTW), np.float32)
    onesmat[0, :] = 1.0
    ident = np.eye(128, dtype=np.float32)

    # x padded to 128 f32 cols
    x_pad = np.zeros((N, F1P), np.float32)
    x_pad[:, :F1] = x

    cfg = dict(
        G_PER_CORE=G_PER_CORE, n_cores=N_CORES,
        N=N, NMAX=NMAX, NPAD_TOT=NPAD_TOT, n_win1=n_win1, n_win2=n_win2,
        T1=T1, T2=T2, n_chunks_x=n_chunks_x, n_chunks_h=n_chunks_h,
        sched1=sched(caps1), sched2=sched(caps2),
        n_idx16_1=int(caps1.sum()) // 16, n_idx16_2=int(caps2.sum()) // 16,
        n_win2_pad16=n_win2_pad16,
        slots1=int(caps1.sum()), slots2=int(caps2.sum()),
    )
    shared = dict(W1aug=W1aug, W2aug=W2aug, W3aug=W3aug, W4aug=W4aug,
                  iota=iota, ident=ident, x_pad=x_pad, onesmat=onesmat)
    per_core = []
    for c in range(N_CORES):
        base1 = np.asarray([b for (b, _) in l1_cores[c]], np.int64)
        base2_arr = np.asarray([b for (b, _) in l2_cores[c]], np.int64)
        per_core.append(dict(
            idx16_1=l1_data[c]["idx16"], dstl1=l1_data[c]["dstl"],
            norm1=l1_data[c]["norm"], meta1=l1_data[c]["meta"],
            idx16_2=l2_data[c]["idx16"], dstl2=l2_data[c]["dstl"],
            norm2=l2_data[c]["norm"], meta2=l2_data[c]["meta"],
            pool_mask=pool_masks[c],
            pool_mask_bcast=np.tile(pool_masks[c][:, None, :], (1, 128, 1)),
            node_start=node_start[c], n_nodes=nodes_per_core[c],
            bases1=base1, bases2=base2_arr,
        ))
    return cfg, per_core, shared


FP32 = mybir.dt.float32
FP32R = mybir.dt.float32r
I16 = mybir.dt.int16
AF = mybir.ActivationFunctionType
ALU = mybir.AluOpType

CHUNK = 32768
DSTW = 256
F1P = 128
F2P = 320
F2S = 384  # h1 stored width (bf16 rows must be 256B multiples)
F2 = 300



def r(ap):
    return ap.bitcast(FP32R)


def build_kernel(cfg, n_cores=8, upto=5, sub=""):
    G = cfg["G_PER_CORE"]
    N, NMAX, NPT = cfg["N"], cfg["NMAX"], cfg["NPAD_TOT"]
    n_win1, n_win2 = cfg["n_win1"], cfg["n_win2"]
    T1, T2 = cfg["T1"], cfg["T2"]
    n_win2p = cfg["n_win2_pad16"]
    sched1, sched2 = cfg["sched1"], cfg["sched2"]

    nc = bacc.Bacc("TRN2", target_bir_lowering=False, debug=False,
                   num_devices=n_cores)

    # ---- I/O ----
    x_pad = nc.dram_tensor("x_pad", [N, F1P], FP32, kind="ExternalInput")
    idx1 = nc.dram_tensor("idx1", [128, cfg["n_idx16_1"]], I16, kind="ExternalInput")
    idx2 = nc.dram_tensor("idx2", [128, cfg["n_idx16_2"]], I16, kind="ExternalInput")
    meta1 = nc.dram_tensor("meta1", [n_win1, 128, 2 * T1], FP32,
                           kind="ExternalInput")
    meta2 = nc.dram_tensor("meta2", [n_win2, 128, 2 * T2], FP32,
                           kind="ExternalInput")
    pmask = nc.dram_tensor("pmask", [G, 128, n_win2p], FP32, kind="ExternalInput")
    w1aug = nc.dram_tensor("w1aug", [80, F2S], FP32, kind="ExternalInput")
    w2aug = nc.dram_tensor("w2aug", [304, F2P], FP32, kind="ExternalInput")
    w3aug = nc.dram_tensor("w3aug", [304, 1024], FP32, kind="ExternalInput")
    w4aug = nc.dram_tensor("w4aug", [1152, 128], FP32, kind="ExternalInput")
    iota_in = nc.dram_tensor("iota", [128, DSTW], FP32, kind="ExternalInput")
    onesmat_in = nc.dram_tensor("onesmat", [128, DSTW], FP32, kind="ExternalInput")
    ident_in = nc.dram_tensor("ident", [128, 128], FP32, kind="ExternalInput")
    z_out = nc.dram_tensor("z", [G, 128], FP32, kind="ExternalOutput")
    if upto == 1:
        dbg1 = nc.dram_tensor("dbg1", [cfg["NMAX"], 80], FP32,
                              kind="ExternalOutput")
    if sub == "gonly":
        dbgg = nc.dram_tensor("dbgg", [128, cfg["T1"] * F1P], FP32,
                              kind="ExternalOutput")
    if upto == 2:
        dbg2 = nc.dram_tensor("dbg2", [cfg["NPAD_TOT"], 80], FP32,
                              kind="ExternalOutput")
    if upto == 3:
        dbg3 = nc.dram_tensor("dbg3", [1024, F2S], FP32, kind="ExternalOutput")

    with tile.TileContext(nc) as tc, \
         tc.tile_pool(name="dram", bufs=1, space="DRAM") as drp, \
         tc.tile_pool(name="consts", bufs=1) as consts:
        # ---- persistent DRAM intermediates ----
        aggx_me = drp.tile([NMAX, 80], FP32, name="aggx_me")
        aggx_full = drp.tile([NPT, 80], FP32, addr_space="Shared",
                             name="aggx_full")
        h1_full = drp.tile([NPT, F2S], mybir.dt.bfloat16, name="h1_full")
        iota_i32 = consts.tile([128, DSTW], mybir.dt.int32)
        nc.gpsimd.iota(iota_i32[:], [[1, DSTW]], base=0, channel_multiplier=0)
        iota_sb = consts.tile([128, DSTW], FP32)
        nc.vector.tensor_copy(iota_sb[:], iota_i32[:])
        ident_sb = consts.tile([128, 128], FP32R)
        nc.sync.dma_start(ident_sb[:], ident_in[:].bitcast(FP32R))
        w1_sb = consts.tile([80, F2S], FP32R)
        nc.sync.dma_start(w1_sb[:], w1aug[:].bitcast(FP32R))
        w2_sb = []
        for k in range(3):
            rows = [128, 128, 48][k]
            t = consts.tile([rows, F2P], FP32R, name=f"w2_sb{k}")
            nc.sync.dma_start(t[:], w2aug[k * 128 : k * 128 + rows, :].bitcast(FP32R))
            w2_sb.append(t)
        w3_sb = []
        for k in range(3):
            rows = [128, 128, 48][k]
            t = consts.tile([rows, 1024], FP32R, name=f"w3_sb{k}")
            nc.sync.dma_start(t[:], w3aug[k * 128 : k * 128 + rows, :].bitcast(FP32R))
            w3_sb.append(t)
        w4_sb = []
        for k in range(9):
            t = consts.tile([128, 128], FP32R, name=f"w4_sb{k}")
            nc.sync.dma_start(t[:], w4aug[k * 128 : (k + 1) * 128, :].bitcast(FP32R))
            w4_sb.append(t)
        w2b_sb = consts.tile([1, F2P], FP32R)
        nc.sync.dma_start(w2b_sb[:], w2aug[300:301, :].bitcast(FP32R))
        w3b_sb = consts.tile([1, 1024], FP32R)
        nc.sync.dma_start(w3b_sb[:], w3aug[300:301, :].bitcast(FP32R))
        ones256_sb = consts.tile([1, DSTW], FP32R)
        nc.sync.dma_start(ones256_sb[:], onesmat_in[0:1, :].bitcast(FP32R))
        # ones tile for the z2 bias chunk: row 0 ones, rest 0
        ones_sb = consts.tile([128, G], FP32R)
        nc.sync.dma_start(ones_sb[:], onesmat_in[:, 0:G].bitcast(FP32R))

        # pooled_win accumulators [128, n_win2p] x3 (feature chunks)
        pooled_win = [consts.tile([128, n_win2p], FP32, name=f"pw{m}")
                      for m in range(3)]
        for m in range(3):
            nc.vector.memset(pooled_win[m][:], -3.0e38)

        # =============== generic window machinery ===============
        def gather_window(pools, w, sched, idx_hbm, table, tbl_rows, T, F, tag,
                          gdt=FP32R, tcast=True):
            gpool, ipool = pools
            ent, tot = sched[w]
            gbuf = gpool.tile([128, T, F], gdt, tag=f"gbuf", name=f"gbuf_{tag}_{w}",
                              padded_shape=[128, T, F])
            c16_0 = ent[0][3]
            c16_n = ent[-1][3] + ent[-1][1] // 16
            itile = ipool.tile([128, c16_n - c16_0], I16, tag="idx",
                               name=f"idx_{tag}_{w}")
            nc.sync.dma_start(itile[:], idx_hbm[:, c16_0:c16_n])
            GMAXI = 1024
            for (k, cap, slot, c16) in ent:
                lo = k * CHUNK
                hi = min(lo + CHUNK, tbl_rows)
                for off in range(0, cap, GMAXI):
                    sub = min(GMAXI, cap - off)
                    so = slot + off
                    co = c16 - c16_0 + off // 16
                    src = table[lo:hi, :]
                    if tcast:
                        src = src.bitcast(gdt)
                    nc.gpsimd.dma_gather(
                        gbuf[:, so // 128 : (so + sub) // 128, :],
                        src,
                        itile[:, co : co + sub // 16],
                        sub, sub, F,
                    )
            return gbuf, tot // 128

        def s_tile(spool, mpool, w, t, dstl_hbm, norm_hbm, T, tag):
            # meta tiles are per-window; load once per window (t==0)
            return None  # built inline below

        # =============== Phase 1: L1 aggregation ===============
        with tc.tile_pool(name="gp1", bufs=2) as gpool, \
             tc.tile_pool(name="ip1", bufs=3) as ipool, \
             tc.tile_pool(name="mp1", bufs=2) as mpool, \
             tc.tile_pool(name="sp1", bufs=4) as spool, \
             tc.tile_pool(name="ps_agg1", bufs=2, space="PSUM") as ps_agg, \
             tc.tile_pool(name="ps_tr1", bufs=2, space="PSUM") as ps_tr, \
             tc.tile_pool(name="sb_ep1", bufs=2) as sb_ep:
            for w in range(n_win1):
                gbuf, nt = gather_window((gpool, ipool), w, sched1, idx1, x_pad,
                                         N, T1, F1P, "l1")
                if sub == "gonly":
                    if w == 0:
                        nc.sync.dma_start(
                            dbgg[:, 0 : nt * F1P],
                            gbuf[:, 0:nt, :].bitcast(FP32))
                    continue
                meta = mpool.tile([128, 2 * T1], FP32, tag="meta", name=f"m1_{w}")
                nc.sync.dma_start(meta[:], meta1[w])
                mab = mpool.tile([1, 1], FP32, tag="mab", name=f"mab1_{w}")
                nc.vector.tensor_copy(mab[:], meta[0:1, 0:1])
                agg = ps_agg.tile([80, DSTW], FP32, tag="agg1", name=f"agg1_{w}")
                for t in range(nt):
                    S = spool.tile([128, DSTW], FP32R, tag="S", name=f"S1_{w}_{t}")
                    nc.vector.tensor_scalar(
                        S[:], iota_sb[:], meta[:, t : t + 1],
                        meta[:, T1 + t : T1 + t + 1], ALU.is_equal, ALU.mult)
                    nc.tensor.matmul(agg[:], gbuf[:, t, 0:80], S[:],
                                     start=(t == 0), stop=(t == nt - 1))
                # epilogue: PSUM [80,256] -> SBUF -> 2x transpose -> aggx_me
                agg_sb = sb_ep.tile([80, DSTW], FP32R, tag="agg_sb", name=f"as1_{w}")
                if sub == "sagg":
                    nc.vector.tensor_copy(agg_sb[:], agg[:])
                else:
                    nc.scalar.activation(agg_sb[:], agg[:], AF.Copy)
                if sub == "noep":
                    continue
                for h in range(2):
                    tp = ps_tr.tile([128, 80], FP32R, tag="tr", name=f"tr1_{w}_{h}")
                    nc.tensor.transpose(tp[:], agg_sb[:, h * 128 : (h + 1) * 128],
                                        ident_sb[0:80, 0:80])
                    osb = sb_ep.tile([128, 80], FP32, tag="osb", name=f"ot1_{w}_{h}")
                    nc.vector.tensor_copy(osb[:], tp[:])
                    if sub != "nomemset":
                        nc.vector.memset(osb[:, 78:79], 1.0)
                    nc.sync.dma_start(
                        aggx_me[w * DSTW + h * 128 : w * DSTW + (h + 1) * 128, :],
                        osb[:])

        # =============== Phase 2: AllGather ===============
        if upto == 1:
            nc.sync.dma_start(dbg1[:], aggx_me[:])
            nc.sync.dma_start(z_out[:, 0:80], aggx_me[0:G, :])
        if upto >= 2:
            nc.gpsimd.collective_compute(
                "AllGather", ALU.bypass,
                replica_groups=[list(range(n_cores))],
                ins=[aggx_me.opt()],
                outs=[aggx_full.opt()],
            )

        # =============== Phase 3: dense h1 ===============
        if upto == 2:
            nc.sync.dma_start(dbg2[:], aggx_full[:])
            nc.sync.dma_start(z_out[:, 0:80], aggx_full[0:G, :])
        with tc.tile_pool(name="ax", bufs=3) as axp, \
             tc.tile_pool(name="ps_tr3", bufs=2, space="PSUM") as ps_tr3, \
             tc.tile_pool(name="ps_h1", bufs=2, space="PSUM") as ps_h1, \
             tc.tile_pool(name="h1sb", bufs=3) as h1sbp:
            for blk in range(NPT // 128 if upto >= 3 else 0):
                a_sb = axp.tile([128, 80], FP32, tag="a", name=f"a3_{blk}")
                nc.sync.dma_start(a_sb[:], aggx_full[blk * 128 : (blk + 1) * 128, :])
                tp = ps_tr3.tile([80, 128], FP32R, tag="tr3", name=f"tr3_{blk}")
                nc.tensor.transpose(tp[:], a_sb[:, 0:80].bitcast(FP32R),
                                    ident_sb[:])
                at_sb = axp.tile([80, 128], FP32R, tag="at", name=f"at3_{blk}")
                nc.vector.tensor_copy(at_sb[:], tp[:])
                hp = ps_h1.tile([128, F2S], FP32, tag="h1p", name=f"h1p_{blk}")
                nc.tensor.matmul(hp[:], at_sb[0:79, :], w1_sb[0:79, :],
                                 start=True, stop=True)
                h1_sb = h1sbp.tile([128, F2S], mybir.dt.bfloat16, tag="h1sb",
                                   name=f"h1sb_{blk}")
                nc.scalar.activation(h1_sb[:], hp[:], AF.Relu)
                nc.sync.dma_start(h1_full[blk * 128 : (blk + 1) * 128, :],
                                  h1_sb[:])

        # =============== Phase 4: L2 aggregation + W2 + window pooling ======
        if upto == 3:
            nc.sync.dma_start(dbg3[:], h1_full[0:1024, :])
            nc.sync.dma_start(z_out[:], h1_full[0:G, 0:128])
        FCH = [(0, 128), (128, 256), (256, 384)]  # gather-feature chunks (lhsT m)
        KCH = [(0, 128), (128, 256), (256, 300)]  # W2 contraction chunks
        with tc.tile_pool(name="gp2", bufs=2) as gpool, \
             tc.tile_pool(name="ip2", bufs=3) as ipool, \
             tc.tile_pool(name="mp2", bufs=2) as mpool, \
             tc.tile_pool(name="sp2", bufs=4) as spool, \
             tc.tile_pool(name="ps_agg2", bufs=2, space="PSUM") as ps_agg2, \
             tc.tile_pool(name="ps_h2", bufs=2, space="PSUM") as ps_h2, \
             tc.tile_pool(name="sb_ep2", bufs=2) as sb_ep2:
            for w in range(n_win2 if upto >= 4 else 0):
                gbuf, nt = gather_window((gpool, ipool), w, sched2, idx2, h1_full,
                                         NPT, T2, F2S, "l2",
                                         gdt=mybir.dt.bfloat16, tcast=False)
                meta = mpool.tile([128, 2 * T2], FP32, tag="meta", name=f"m2_{w}")
                nc.sync.dma_start(meta[:], meta2[w])
                mab = mpool.tile([1, 1], FP32, tag="mab", name=f"mab2_{w}")
                nc.vector.tensor_copy(mab[:], meta[0:1, 0:1])
                aggs = []
                for fi, (f0, f1) in enumerate(FCH):
                    aggs.append(ps_agg2.tile([f1 - f0, DSTW], FP32,
                                             tag=f"agg2_{fi}", name=f"agg2_{w}_{fi}"))
                for t in range(nt):
                    S = spool.tile([128, DSTW], mybir.dt.bfloat16, tag="S",
                                   name=f"S2_{w}_{t}")
                    nc.vector.tensor_scalar(
                        S[:], iota_sb[:], meta[:, t : t + 1],
                        meta[:, T2 + t : T2 + t + 1], ALU.is_equal, ALU.mult)
                    for fi, (f0, f1) in enumerate(FCH):
                        nc.tensor.matmul(aggs[fi][:], gbuf[:, t, f0:f1], S[:],
                                         start=(t == 0), stop=(t == nt - 1))
                # copy agg chunks to SBUF; chunk2 gets the ones row at 300-256=44
                a_sb = []
                for fi, (f0, f1) in enumerate(FCH):
                    t_ = sb_ep2.tile([f1 - f0, DSTW], FP32R, tag=f"a2sb_{fi}",
                                     name=f"a2sb_{w}_{fi}")
                    nc.scalar.activation(t_[:], aggs[fi][:], AF.Copy)
                    a_sb.append(t_)
                for m, (m0, m1) in enumerate([(0, 128), (128, 256), (256, 300)]):
                    hp = ps_h2.tile([m1 - m0, DSTW], FP32, tag="h2p",
                                    name=f"h2p_{w}_{m}")
                    for ki, (k0, k1) in enumerate(KCH):
                        nc.tensor.matmul(
                            hp[:], w2_sb[ki][0 : k1 - k0, m0:m1],
                            a_sb[ki][0 : k1 - k0, :],
                            start=(ki == 0), stop=False)
                    nc.tensor.matmul(hp[:], w2b_sb[:, m0:m1], ones256_sb[:],
                                     start=False, stop=True)
                    h2sb = sb_ep2.tile([m1 - m0, DSTW], FP32, tag="h2sb",
                                       name=f"h2sb_{w}_{m}")
                    nc.scalar.activation(h2sb[:], hp[:], AF.Relu)
                    nc.vector.tensor_reduce(
                        pooled_win[m][0 : m1 - m0, w : w + 1], h2sb[:],
                        axis=mybir.AxisListType.X, op=ALU.max)

        # =============== Phase 5: pool combine + MLP ===============
        if upto >= 4:
         with tc.tile_pool(name="pm", bufs=3) as pmp, \
              tc.tile_pool(name="pool5", bufs=2) as p5, \
              tc.tile_pool(name="ps_z", bufs=2, space="PSUM") as psz, \
              tc.tile_pool(name="zsb", bufs=2) as zsb:
             pooledT = [p5.tile([128, G], FP32, tag=f"pT{m}", bufs=1,
                                name=f"pooledT{m}") for m in range(3)]
             for g in range(G):
                 msk = pmp.tile([128, n_win2p], FP32, tag="msk", name=f"msk_{g}")
                 nc.sync.dma_start(msk[:], pmask[g])
                 for m in range(3):
                     tmp = pmp.tile([128, n_win2p], FP32, tag="tmp",
                                    name=f"tmp_{g}_{m}")
                     nc.vector.tensor_tensor(tmp[:], pooled_win[m][:], msk[:],
                                             ALU.add)
                     nc.vector.tensor_reduce(
                         pooledT[m][:, g : g + 1], tmp[:],
                         axis=mybir.AxisListType.X, op=ALU.max)
             # relu (empty graphs -> 0) then ones row at global 300 (chunk2 row 44)
             pooledTr = [p5.tile([128, G], FP32R, tag=f"pTr{m}", bufs=1,
                                 name=f"pooledTr{m}") for m in range(3)]
             for m in range(3):
                 nc.scalar.activation(pooledTr[m][:], pooledT[m][:], AF.Relu)
             # z1T [1024 (8 chunks), 64]
             z1t = []
             for mi in range(8):
                 zp = psz.tile([128, G], FP32, tag="z1p", name=f"z1p_{mi}")
                 for ki, (k0, k1) in enumerate(KCH):
                     nc.tensor.matmul(
                         zp[:], w3_sb[ki][0 : k1 - k0, mi * 128 : (mi + 1) * 128],
                         pooledTr[ki][0 : k1 - k0, :],
                         start=(ki == 0), stop=False)
                 nc.tensor.matmul(zp[:], w3b_sb[:, mi * 128 : (mi + 1) * 128],
                                  ones_sb[0:1, :], start=False, stop=True)
                 zt = zsb.tile([128, G], FP32R, tag=f"z1t{mi}", bufs=1,
                               name=f"z1t_{mi}")
                 nc.scalar.activation(zt[:], zp[:], AF.Relu)
                 z1t.append(zt)
             zp2 = psz.tile([G, 128], FP32, tag="z2p", name="z2p")
             for ki in range(9):
                 lhsT = z1t[ki][:] if ki < 8 else ones_sb[:]
                 nc.tensor.matmul(zp2[:], lhsT, w4_sb[ki][:],
                                  start=(ki == 0), stop=(ki == 8))
             zfin = zsb.tile([G, 128], FP32, tag="zfin", name="zfin")
             nc.scalar.activation(zfin[:], zp2[:], AF.Relu)
             nc.sync.dma_start(z_out[:], zfin[:])

    nc.compile()
    nc.generate_event_semaphores()
    return nc


# ======================= public entry point =======================
_NC_CACHE = {}


def _make_in_maps(cfg, per_core, shared):
    base = dict(
        x_pad=shared["x_pad"], w1aug=shared["W1aug"], w2aug=shared["W2aug"],
        w3aug=shared["W3aug"], w4aug=shared["W4aug"], iota=shared["iota"],
        ident=shared["ident"], onesmat=shared["onesmat"],
    )
    maps = []
    for pc in per_core:
        m = dict(base)
        m["idx1"] = pc["idx16_1"]
        m["idx2"] = pc["idx16_2"]
        m["meta1"] = pc["meta1"]
        m["meta2"] = pc["meta2"]
        m["pmask"] = pc["pool_mask_bcast"]
        maps.append(m)
    return maps


def kernel(x, edge_index, batch, W1, b1, W2, b2, W3, b3, W4, b4,
           trace=False):
    weights = dict(W1=np.asarray(W1, np.float32), b1=np.asarray(b1, np.float32),
                   W2=np.asarray(W2, np.float32), b2=np.asarray(b2, np.float32),
                   W3=np.asarray(W3, np.float32), b3=np.asarray(b3, np.float32),
                   W4=np.asarray(W4, np.float32), b4=np.asarray(b4, np.float32))
    n_graphs = 512
    n_cores = 8
    cfg, per_core, shared = build_plan(
        np.asarray(x, np.float32), np.asarray(edge_index), np.asarray(batch),
        weights, n_graphs=n_graphs, n_cores=n_cores)
    key = (cfg["N"], cfg["NMAX"], cfg["n_win1"], cfg["n_win2"], cfg["T1"],
           cfg["T2"], cfg["n_idx16_1"], cfg["n_idx16_2"], cfg["n_win2_pad16"])
    if key not in _NC_CACHE:
        _NC_CACHE[key] = build_kernel(cfg, n_cores=n_cores)
    nc = _NC_CACHE[key]
    maps = _make_in_maps(cfg, per_core, shared)
    res = run_bass_kernel_spmd(nc, maps, core_ids=list(range(n_cores)),
                               trace=trace)
    z = np.concatenate([res.results[c]["z"] for c in range(n_cores)], axis=0)
    if trace:
        kernel.last_results = res
    return z.astype(np.float32)



# revision 13
# speedup vs baseline: 1.3594x; 1.3594x over previous
"""Trainium2 Bass kernel for nn_DrugGCNncoder (2-layer GCN + max-pool + MLP).

Self-contained: accepts the FULL inputs of reference.setup_inputs(), shards
across 8 NeuronCores internally (dst-node/graph sharding), returns the FULL
[512, 128] output.

v2 design (vs v1 baseline):
 - bf16 gather tables, S-matrices and weights (2x DVE, half gather bytes).
 - W1 + relu + W2 fused per-node into the L1 window epilogue, producing
   z = relu(agg@W1+b1)@W2 directly; AllGather ships z (no dense phase 3,
   no transposes anywhere).
 - L1 self-loops folded into the epilogue via a host-precomputed
   norm_self * x^T tensor (removes them from the gather).
 - Window max-pool on the RAW aggregate; bias+relu applied after pooling
   (exact because relu is monotone; empty graphs correct because b2 == 0).
 - Graph-uniform window slots (Wmax per graph) -> compile-time segment
   reduce, no pooling masks.
 - Index padding with -1 sentinels: the gather ucode skips trailing -1s,
   so padded slots cost zero descriptors on each core.
 - 4 SWDGE queues, gather calls rotate across them.
 - AllGather split into 4 chunks, issued with one-group lag to overlap
   the transfer with the L1 tail.
"""
import sys
for p in ("/opt/trn_rl_repo", "/root/.axon_site/_ro/trn_rl_repo"):
    if p not in sys.path:
        sys.path.insert(0, p)
import numpy as np
import concourse.bass as bass
import concourse.bacc as bacc
import concourse.mybir as mybir
from concourse import tile
from concourse.bass_utils import run_bass_kernel_spmd

FP32 = mybir.dt.float32
BF16 = mybir.dt.bfloat16
I16 = mybir.dt.int16
AF = mybir.ActivationFunctionType
ALU = mybir.AluOpType

CHUNK_X = 32768      # x-table chunk rows (int16 index range)
DSTW = 256           # window width in dst-node columns
F1P = 128            # x padded feature count (bf16 -> 256B rows)
F2P = 384            # z padded feature count (bf16 -> 768B rows)
F1 = 78
F2 = 300
FOUT = 128
N_CORES = 8
N_GRAPHS = 512
GMAXI = 1024         # max rows per dma_gather call
NQ = 1               # SWDGE queues


def _pack_idx16(idx, cap):
    """idx (valid list) -> [128, cap//16] int16, slot j at [j%16, j//16],
    padded with -1 (skipped by the gather ucode), replicated 8x."""
    assert cap % 16 == 0 and len(idx) <= cap
    full = np.full(cap, -1, np.int16)
    full[: len(idx)] = idx
    blk = full.reshape(cap // 16, 16).T  # [16, cap/16]
    return np.tile(blk, (8, 1))  # [128, cap/16]


def build_plan(x, edge_index, batch, weights, n_graphs=512, n_cores=8):
    N = x.shape[0]
    G = n_graphs // n_cores
    src = edge_index[0].astype(np.int64)
    dst = edge_index[1].astype(np.int64)
    deg = (np.bincount(dst, minlength=N) + 1).astype(np.float64)  # + self loop
    dis = 1.0 / np.sqrt(deg)
    norm_e = (dis[src] * dis[dst]).astype(np.float32)
    norm_self = (dis * dis).astype(np.float32)

    batch = batch.astype(np.int64)
    g_start = np.searchsorted(batch, np.arange(n_graphs), side="left")
    g_end = np.searchsorted(batch, np.arange(n_graphs), side="right")
    node_start = [int(g_start[c * G]) for c in range(n_cores)]
    node_start.append(N)
    nodes_per_core = [node_start[c + 1] - node_start[c] for c in range(n_cores)]
    NMAX = ((max(nodes_per_core) + DSTW - 1) // DSTW) * DSTW
    n_win1 = NMAX // DSTW

    core_of = np.searchsorted(np.asarray(node_start[1:]), np.arange(N),
                              side="right")
    local_of = np.arange(N) - np.asarray(node_start)[core_of]

    # ---- z-table chunking (for chunked AllGather + int16 range) ----------
    ngrp = 4
    base_w = n_win1 // ngrp
    extra = n_win1 - base_w * ngrp
    grp_sizes = [base_w + (1 if j < extra else 0) for j in range(ngrp)]
    grp_w0 = np.cumsum([0] + grp_sizes)          # window offsets, len 5
    grp_rows = [s * DSTW for s in grp_sizes]     # local rows per group
    grp_r0 = np.cumsum([0] + grp_rows)           # local row offsets, len 5
    assert all(8 * r <= 32768 for r in grp_rows)
    # map local row -> (group j, row within z_full[j]) for a given core
    grp_of_local = np.searchsorted(grp_r0[1:], np.arange(NMAX), side="right")

    # ---- per-core dst-sorted edges --------------------------------------
    per_core_raw = []
    for c in range(n_cores):
        sel = (dst >= node_start[c]) & (dst < node_start[c + 1])
        s, d, nm = src[sel], dst[sel], norm_e[sel]
        dl = d - node_start[c]
        order = np.argsort(dl, kind="stable")
        per_core_raw.append((s[order], dl[order], nm[order]))

    # ---- L2 graph-uniform windows ---------------------------------------
    g_len = (g_end - g_start).astype(np.int64)
    Wmax = max(1, int((g_len.max() + DSTW - 1) // DSTW))
    n_win2 = G * Wmax

    # L1: source row in x table; chunk by global id // 32768
    n_chunks_x = (N + CHUNK_X - 1) // CHUNK_X
    # L2: source row in z_full[j]; j from the SOURCE node's local offset
    src_grp = grp_of_local[np.minimum(local_of, NMAX - 1)]
    src_zrow = (core_of * np.asarray(grp_rows)[src_grp]
                + (local_of - np.asarray(grp_r0)[src_grp]))
    n_chunks_z = ngrp

    def windows_l1(c):
        s_loc, dl, nm = per_core_raw[c]
        out = []
        for w in range(n_win1):
            lo = np.searchsorted(dl, w * DSTW, side="left")
            hi = np.searchsorted(dl, (w + 1) * DSTW, side="left")
            es, edl, enm = s_loc[lo:hi], dl[lo:hi] - w * DSTW, nm[lo:hi]
            ch = es // CHUNK_X
            runs = []
            for k in range(n_chunks_x):
                m = ch == k
                runs.append((es[m] - k * CHUNK_X, edl[m], enm[m]))
            out.append(runs)
        return out

    def windows_l2(c):
        s_loc, dl, nm = per_core_raw[c]
        # self-loop edges for this core's own nodes (kept in the L2 gather)
        own = np.arange(node_start[c], node_start[c + 1])
        sl_dl = own - node_start[c]
        all_src = np.concatenate([s_loc, own])
        all_dl = np.concatenate([dl, sl_dl])
        all_nm = np.concatenate([nm, norm_self[own]]).astype(np.float32)
        order = np.argsort(all_dl, kind="stable")
        all_src, all_dl, all_nm = all_src[order], all_dl[order], all_nm[order]
        out = []
        for gl in range(G):
            g = c * G + gl
            glo = int(g_start[g] - node_start[c])
            ghi = int(g_end[g] - node_start[c])
            for swin in range(Wmax):
                base = glo + swin * DSTW
                top = min(base + DSTW, ghi)
                lo = np.searchsorted(all_dl, base, side="left")
                hi = np.searchsorted(all_dl, max(top, base), side="left")
                es = all_src[lo:hi]
                edl = all_dl[lo:hi] - base
                enm = all_nm[lo:hi]
                rows = src_zrow[es]
                ch = src_grp[es] if len(es) else np.zeros(0, np.int64)
                runs = []
                for k in range(n_chunks_z):
                    m = ch == k
                    runs.append((rows[m], edl[m], enm[m]))
                out.append(runs)
        return out

    l1_cores = [windows_l1(c) for c in range(n_cores)]
    l2_cores = [windows_l2(c) for c in range(n_cores)]

    def normalize(cores_wins, n_win, n_chunks, force_first=False):
        caps = np.zeros((n_win, n_chunks), np.int64)
        for wins in cores_wins:
            for w in range(n_win):
                for k in range(n_chunks):
                    caps[w, k] = max(caps[w, k], len(wins[w][k][0]))
        caps = ((caps + 127) // 128) * 128
        if force_first:
            caps[:, 0] = np.maximum(caps[:, 0], 128)
        T = int(caps.sum(axis=1).max()) // 128
        return caps, T

    caps1, T1 = normalize(l1_cores, n_win1, n_chunks_x, force_first=True)
    caps2, T2 = normalize(l2_cores, n_win2, n_chunks_z, force_first=True)

    def emit(cores_wins, caps, n_win, T, n_chunks):
        n_idx16 = int(caps.sum()) // 16
        out = []
        for wins in cores_wins:
            idx16 = np.full((128, n_idx16), -1, np.int16)
            meta = np.zeros((n_win, 128, 2 * T), np.float32)
            meta[:, :, :T] = -1.0  # dstl pad
            col16 = 0
            for w in range(n_win):
                slot = 0
                for k in range(n_chunks):
                    cap = int(caps[w, k])
                    ri, rd, rn = wins[w][k]
                    idx16[:, col16 : col16 + cap // 16] = _pack_idx16(ri, cap)
                    n = len(ri)
                    sl = slot + np.arange(n)
                    meta[w, sl % 128, sl // 128] = rd.astype(np.float32)
                    meta[w, sl % 128, T + sl // 128] = rn
                    slot += cap
                    col16 += cap // 16
                assert slot <= T * 128
            out.append({"idx16": idx16,
                        "meta": meta.astype(np.float32)})  # cast to bf16 later
        return out

    l1_data = emit(l1_cores, caps1, n_win1, T1, n_chunks_x)
    l2_data = emit(l2_cores, caps2, n_win2, T2, n_chunks_z)

    def call_counts(cores_wins, caps, n_win, n_chunks):
        """Per-core valid-index count for every dma_gather call, in issue
        order (w, then k with cap>0, then GMAXI sub-calls)."""
        out = []
        for wins in cores_wins:
            cnts = []
            for w in range(n_win):
                for k in range(n_chunks):
                    cap = int(caps[w, k])
                    if cap == 0:
                        continue
                    nvalid = len(wins[w][k][0])
                    for off in range(0, cap, GMAXI):
                        sub = min(GMAXI, cap - off)
                        cnts.append(max(0, min(sub, nvalid - off)))
            out.append(np.asarray(cnts, np.int32).reshape(1, -1))
        return out

    cnt1 = call_counts(l1_cores, caps1, n_win1, n_chunks_x)
    cnt2 = call_counts(l2_cores, caps2, n_win2, n_chunks_z)

    def sched(caps):
        rows = []
        col16 = 0
        for w in range(caps.shape[0]):
            slot = 0
            ent = []
            for k in range(caps.shape[1]):
                cap = int(caps[w, k])
                if cap > 0:
                    ent.append((k, cap, slot, col16))
                slot += cap
                col16 += cap // 16
            rows.append((ent, slot))
        return rows

    # ---- packed weights (bf16) ------------------------------------------
    W1, b1, W2, b2, W3, b3, W4, b4 = (
        weights["W1"], weights["b1"], weights["W2"], weights["b2"],
        weights["W3"], weights["b3"], weights["W4"], weights["b4"],
    )
    w1aug = np.zeros((80, F2P), np.float32)
    w1aug[:F1, :F2] = W1
    w1aug[F1, :F2] = b1       # ones-row slot 78
    w2aug = np.zeros((F2P, F2P), np.float32)
    w2aug[:F2, :F2] = W2
    w3aug = np.zeros((F2P, 1024), np.float32)
    w3aug[:F2, :] = W3
    w4aug = np.zeros((1024, FOUT), np.float32)
    w4aug[:, :] = W4
    b4row = b4.reshape(1, FOUT).astype(np.float32)
    biases = np.zeros((128, 11), np.float32)
    for m in range(3):
        seg = np.zeros(128, np.float32)
        seg[: max(0, min(128, F2 - m * 128))] = b2[m * 128 : (m + 1) * 128]
        biases[:, m] = seg
    for m in range(8):
        biases[:, 3 + m] = b3[m * 128 : (m + 1) * 128]

    # x table bf16 [N, 128]
    x_bf = np.zeros((N, F1P), np.float32)
    x_bf[:, :F1] = x

    # per-core norm_self * x^T with ones row at 78
    xtn = []
    for c in range(n_cores):
        t = np.zeros((80, NMAX), np.float32)
        nn = nodes_per_core[c]
        own = np.arange(node_start[c], node_start[c + 1])
        t[:F1, :nn] = (x[own] * norm_self[own][:, None]).T
        t[F1, :] = 1.0
        xtn.append(t)

    cfg = dict(
        N=N, G=G, NMAX=NMAX, n_win1=n_win1, n_win2=n_win2, Wmax=Wmax,
        T1=T1, T2=T2, n_chunks_x=n_chunks_x, n_chunks_z=n_chunks_z,
        sched1=sched(caps1), sched2=sched(caps2),
        n_idx16_1=int(caps1.sum()) // 16, n_idx16_2=int(caps2.sum()) // 16,
        grp_sizes=grp_sizes, grp_rows=grp_rows,
        grp_w0=[int(v) for v in grp_w0], grp_r0=[int(v) for v in grp_r0],
        n_cores=n_cores, n_calls1=cnt1[0].shape[1], n_calls2=cnt2[0].shape[1],
    )
    shared = dict(x_bf=x_bf, w1aug=w1aug, w2aug=w2aug, w3aug=w3aug,
                  w4aug=w4aug, b4row=b4row, biases=biases)
    per_core = []
    for c in range(n_cores):
        per_core.append(dict(
            idx1=l1_data[c]["idx16"], meta1=l1_data[c]["meta"],
            idx2=l2_data[c]["idx16"], meta2=l2_data[c]["meta"],
            xtn=xtn[c], cnt1=cnt1[c], cnt2=cnt2[c],
        ))
    return cfg, per_core, shared


def build_kernel(cfg, n_cores=8, upto=5):
    G = cfg["G"]
    NMAX, n_win1, n_win2 = cfg["NMAX"], cfg["n_win1"], cfg["n_win2"]
    Wmax = cfg["Wmax"]
    T1, T2 = cfg["T1"], cfg["T2"]
    sched1, sched2 = cfg["sched1"], cfg["sched2"]
    grp_rows, grp_w0, grp_r0 = cfg["grp_rows"], cfg["grp_w0"], cfg["grp_r0"]
    ngrp = len(grp_rows)

    nc = bacc.Bacc("TRN2", target_bir_lowering=False, debug=False,
                   num_devices=n_cores, num_swdge_queues=NQ)

    # ---- I/O ----
    x_bf = nc.dram_tensor("x_bf", [cfg["N"], F1P], BF16, kind="ExternalInput")
    xtn_in = nc.dram_tensor("xtn", [80, NMAX], FP32, kind="ExternalInput")
    idx1 = nc.dram_tensor("idx1", [128, cfg["n_idx16_1"]], I16,
                          kind="ExternalInput")
    idx2 = nc.dram_tensor("idx2", [128, cfg["n_idx16_2"]], I16,
                          kind="ExternalInput")
    meta1 = nc.dram_tensor("meta1", [n_win1, 128, 2 * T1], FP32,
                           kind="ExternalInput")
    meta2 = nc.dram_tensor("meta2", [n_win2, 128, 2 * T2], FP32,
                           kind="ExternalInput")
    w1_in = nc.dram_tensor("w1aug", [80, F2P], BF16, kind="ExternalInput")
    w2_in = nc.dram_tensor("w2aug", [F2P, F2P], BF16, kind="ExternalInput")
    w3_in = nc.dram_tensor("w3aug", [F2P, 1024], BF16, kind="ExternalInput")
    w4_in = nc.dram_tensor("w4aug", [1024, FOUT], BF16, kind="ExternalInput")
    b4_in = nc.dram_tensor("b4row", [1, FOUT], BF16, kind="ExternalInput")
    bias_in = nc.dram_tensor("biases", [128, 11], FP32, kind="ExternalInput")
    cnt1_in = nc.dram_tensor("cnt1", [1, cfg["n_calls1"]], mybir.dt.int32,
                             kind="ExternalInput")
    cnt2_in = nc.dram_tensor("cnt2", [1, cfg["n_calls2"]], mybir.dt.int32,
                             kind="ExternalInput")
    z_out = nc.dram_tensor("z", [G, FOUT], FP32, kind="ExternalOutput")
    if upto == 1:
        dbg1 = nc.dram_tensor("dbg1", [NMAX, F2P], BF16, kind="ExternalOutput")
    if upto == 2:
        dbg2 = nc.dram_tensor("dbg2", [8 * grp_rows[0], F2P], BF16,
                              kind="ExternalOutput")

    with tile.TileContext(nc) as tc, \
         tc.tile_pool(name="dram", bufs=1, space="DRAM") as drp, \
         tc.tile_pool(name="consts", bufs=1) as consts:
        z_me = drp.tile([NMAX, F2P], BF16, name="z_me")
        z_full = [drp.tile([n_cores * grp_rows[j], F2P], BF16,
                           addr_space="Shared", name=f"z_full{j}")
                  for j in range(ngrp)]

        iota_i32 = consts.tile([128, DSTW], mybir.dt.int32)
        nc.gpsimd.iota(iota_i32[:], [[1, DSTW]], base=0, channel_multiplier=0)
        iota_bf = consts.tile([128, DSTW], BF16)
        nc.vector.tensor_copy(iota_bf[:], iota_i32[:])
        w1_sb = consts.tile([80, F2P], BF16)
        nc.sync.dma_start(w1_sb[:], w1_in[:])
        w2_sb = [consts.tile([128, F2P], BF16, name=f"w2_{k}") for k in range(3)]
        for k in range(3):
            nc.sync.dma_start(w2_sb[k][:], w2_in[k * 128 : (k + 1) * 128, :])
        w3_sb = [consts.tile([128, 1024], BF16, name=f"w3_{k}") for k in range(3)]
        for k in range(3):
            nc.sync.dma_start(w3_sb[k][:], w3_in[k * 128 : (k + 1) * 128, :])
        w4_sb = [consts.tile([128, FOUT], BF16, name=f"w4_{k}") for k in range(8)]
        for k in range(8):
            nc.sync.dma_start(w4_sb[k][:], w4_in[k * 128 : (k + 1) * 128, :])
        b4_sb = consts.tile([1, FOUT], BF16)
        nc.sync.dma_start(b4_sb[:], b4_in[:])
        bias_sb = consts.tile([128, 11], FP32)
        nc.sync.dma_start(bias_sb[:], bias_in[:])
        ones64 = consts.tile([1, G], BF16)
        nc.vector.memset(ones64[:], 1.0)
        cnt1_sb = consts.tile([1, cfg["n_calls1"]], mybir.dt.int32)
        nc.sync.dma_start(cnt1_sb[:], cnt1_in[:])
        cnt2_sb = consts.tile([1, cfg["n_calls2"]], mybir.dt.int32)
        nc.sync.dma_start(cnt2_sb[:], cnt2_in[:])
        cnt_regs = [nc.gpsimd.alloc_register(f"cnt_reg{i}") for i in range(4)]
        pooled_win = [consts.tile([128, G, Wmax], FP32, name=f"pw{m}")
                      for m in range(3)]

        qc = [0]

        def gather_window(gpool, ipool, w, sched, idx_hbm, tables, T, F, tag,
                          memset_first, cnt_sb, call_i):
            ent, tot = sched[w]
            gbuf = gpool.tile([128, T, F], BF16, tag="gbuf",
                              name=f"gbuf_{tag}_{w}", padded_shape=[128, T, F])
            if memset_first:
                nc.vector.memset(gbuf[:], 0.0)
            c16_0 = ent[0][3]
            c16_n = ent[-1][3] + ent[-1][1] // 16
            itile = ipool.tile([128, c16_n - c16_0], I16, tag="idx",
                               name=f"idx_{tag}_{w}")
            nc.sync.dma_start(itile[:], idx_hbm[:, c16_0:c16_n])
            for (k, cap, slot, c16) in ent:
                table = tables[k]
                for off in range(0, cap, GMAXI):
                    sub = min(GMAXI, cap - off)
                    so = slot + off
                    co = c16 - c16_0 + off // 16
                    ci = call_i[0]
                    call_i[0] += 1
                    nval = cnt_regs[ci % 4]
                    nc.gpsimd.reg_load(nval, cnt_sb[0:1, ci : ci + 1])
                    nc.gpsimd.dma_gather(
                        gbuf[:, so // 128 : (so + sub) // 128, :],
                        table,
                        itile[:, co : co + sub // 16],
                        sub, nval, F,
                        queue_num=qc[0] % NQ,
                    )
                    qc[0] += 1
            return gbuf, tot // 128

        # =============== Phase 1: L1 windows + fused node transform =======
        x_tables = [x_bf[k * CHUNK_X : min((k + 1) * CHUNK_X, cfg["N"]), :]
                    for k in range(cfg["n_chunks_x"])]
        with tc.tile_pool(name="gp1", bufs=2) as gpool, \
             tc.tile_pool(name="ip1", bufs=3) as ipool, \
             tc.tile_pool(name="mp1", bufs=2) as mpool, \
             tc.tile_pool(name="sp1", bufs=4) as spool, \
             tc.tile_pool(name="sb1", bufs=3) as sbp, \
             tc.tile_pool(name="ps_agg", bufs=2, space="PSUM") as psA, \
             tc.tile_pool(name="ps_h1", bufs=2, space="PSUM") as psB, \
             tc.tile_pool(name="ps_z", bufs=2, space="PSUM") as psC:
            pending_cc = []
            call1 = [0]
            for j in range(ngrp):
                for w in range(grp_w0[j], grp_w0[j + 1]):
                    gbuf, nt = gather_window(gpool, ipool, w, sched1, idx1,
                                             x_tables, T1, F1P, "l1", w < 2,
                                             cnt1_sb, call1)
                    meta = mpool.tile([128, 2 * T1], FP32, tag="meta",
                                      name=f"m1_{w}")
                    nc.sync.dma_start(meta[:], meta1[w])
                    xw = mpool.tile([80, DSTW], FP32, tag="xtn",
                                    name=f"xw_{w}")
                    nc.sync.dma_start(
                        xw[:], xtn_in[:, w * DSTW : (w + 1) * DSTW])
                    agg = psA.tile([80, DSTW], FP32, tag="agg",
                                   name=f"agg_{w}")
                    for t in range(nt):
                        S = spool.tile([128, DSTW], BF16, tag="S",
                                       name=f"S1_{w}_{t}")
                        nc.vector.tensor_scalar(
                            S[:], iota_bf[:], meta[:, t : t + 1],
                            meta[:, T1 + t : T1 + t + 1],
                            ALU.is_equal, ALU.mult)
                        nc.tensor.matmul(agg[:], gbuf[:, t, 0:80], S[:],
                                         start=(t == 0), stop=(t == nt - 1))
                    asb = sbp.tile([80, DSTW], BF16, tag="asb",
                                   name=f"asb_{w}")
                    nc.vector.tensor_tensor(asb[:], agg[:], xw[:], ALU.add)
                    zp = [psC.tile([128, F2P], FP32, tag=f"zp{h}",
                                   name=f"zp_{w}_{h}") for h in range(2)]
                    for ki in range(3):
                        hp = psB.tile([128, DSTW], FP32, tag="hp",
                                      name=f"hp_{w}_{ki}")
                        nc.tensor.matmul(
                            hp[:], w1_sb[0:79, ki * 128 : (ki + 1) * 128],
                            asb[0:79, :], start=True, stop=True)
                        ht = sbp.tile([128, DSTW], BF16, tag="ht",
                                      name=f"ht_{w}_{ki}")
                        nc.scalar.activation(ht[:], hp[:], AF.Relu)
                        for h in range(2):
                            nc.tensor.matmul(
                                zp[h][:], ht[:, h * 128 : (h + 1) * 128],
                                w2_sb[ki][:], start=(ki == 0), stop=(ki == 2))
                    for h in range(2):
                        zsb = sbp.tile([128, F2P], BF16, tag="zsb",
                                       name=f"zsb_{w}_{h}")
                        nc.scalar.activation(zsb[:], zp[h][:], AF.Copy)
                        nc.sync.dma_start(
                            z_me[w * DSTW + h * 128 : w * DSTW + (h + 1) * 128,
                                 :], zsb[:])
                # lag-one-group collective issue to avoid stalling gathers
                pending_cc.append(j)
                if upto >= 2 and len(pending_cc) > 1:
                    jj = pending_cc.pop(0)
                    nc.gpsimd.collective_compute(
                        "AllGather", ALU.bypass,
                        replica_groups=[list(range(n_cores))],
                        ins=[z_me[grp_r0[jj] : grp_r0[jj + 1], :].opt()],
                        outs=[z_full[jj][:].opt()],
                    )
            for jj in (pending_cc if upto >= 2 else []):
                nc.gpsimd.collective_compute(
                    "AllGather", ALU.bypass,
                    replica_groups=[list(range(n_cores))],
                    ins=[z_me[grp_r0[jj] : grp_r0[jj + 1], :].opt()],
                    outs=[z_full[jj][:].opt()],
                )

        if upto == 1:
            nc.sync.dma_start(dbg1[:], z_me[:])
        if upto == 2:
            nc.sync.dma_start(dbg2[:], z_full[0][:])

        # =============== Phase 2: L2 windows + raw-agg pooling =============
        z_tables = [z_full[k][:] for k in range(ngrp)]
        with tc.tile_pool(name="gp2", bufs=2) as gpool, \
             tc.tile_pool(name="ip2", bufs=3) as ipool, \
             tc.tile_pool(name="mp2", bufs=2) as mpool, \
             tc.tile_pool(name="sp2", bufs=4) as spool, \
             tc.tile_pool(name="ps_a2", bufs=2, space="PSUM") as ps2:
            call2 = [0]
            for w in range(n_win2 if upto >= 4 else 0):
                gbuf, nt = gather_window(gpool, ipool, w, sched2, idx2,
                                         z_tables, T2, F2P, "l2", w < 2,
                                         cnt2_sb, call2)
                meta = mpool.tile([128, 2 * T2], FP32, tag="meta",
                                  name=f"m2_{w}")
                nc.sync.dma_start(meta[:], meta2[w])
                aggs = [ps2.tile([128, DSTW], FP32, tag=f"a2_{fi}",
                                 name=f"a2_{w}_{fi}") for fi in range(3)]
                for t in range(nt):
                    S = spool.tile([128, DSTW], BF16, tag="S",
                                   name=f"S2_{w}_{t}")
                    nc.vector.tensor_scalar(
                        S[:], iota_bf[:], meta[:, t : t + 1],
                        meta[:, T2 + t : T2 + t + 1], ALU.is_equal, ALU.mult)
                    for fi in range(3):
                        nc.tensor.matmul(
                            aggs[fi][:], gbuf[:, t, fi * 128 : (fi + 1) * 128],
                            S[:], start=(t == 0), stop=(t == nt - 1))
                gl, sw = w // Wmax, w % Wmax
                for fi in range(3):
                    nc.vector.tensor_reduce(
                        pooled_win[fi][:, gl, sw : sw + 1], aggs[fi][:],
                        axis=mybir.AxisListType.X, op=ALU.max)

        # =============== Phase 3: pool combine + MLP =======================
        if upto >= 4:
            with tc.tile_pool(name="p5", bufs=2) as p5, \
                 tc.tile_pool(name="ps_mlp", bufs=4, space="PSUM") as psz, \
                 tc.tile_pool(name="zsb5", bufs=1) as zsbp:
                pooledTr = []
                for m in range(3):
                    praw = p5.tile([128, G], FP32, tag="praw",
                                   name=f"praw{m}")
                    nc.vector.tensor_reduce(
                        praw[:], pooled_win[m][:],
                        axis=mybir.AxisListType.X, op=ALU.max)
                    pr = zsbp.tile([128, G], BF16, name=f"pTr{m}")
                    nc.scalar.activation(pr[:], praw[:], AF.Relu,
                                         bias=bias_sb[:, m : m + 1])
                    pooledTr.append(pr)
                z1t = []
                for mi in range(8):
                    zp = psz.tile([128, G], FP32, tag="z1p",
                                  name=f"z1p_{mi}")
                    for ki in range(3):
                        nc.tensor.matmul(
                            zp[:],
                            w3_sb[ki][:, mi * 128 : (mi + 1) * 128],
                            pooledTr[ki][:], start=(ki == 0), stop=(ki == 2))
                    zt = zsbp.tile([128, G], BF16, name=f"z1t_{mi}")
                    nc.scalar.activation(zt[:], zp[:], AF.Relu,
                                         bias=bias_sb[:, 3 + mi : 4 + mi])
                    z1t.append(zt)
                zp2 = psz.tile([G, FOUT], FP32, tag="z2p", name="z2p")
                for ki in range(9):
                    lhsT = z1t[ki][:] if ki < 8 else ones64[:]
                    rhs = w4_sb[ki][:] if ki < 8 else b4_sb[:]
                    nc.tensor.matmul(zp2[:], lhsT, rhs,
                                     start=(ki == 0), stop=(ki == 8))
                zfin = zsbp.tile([G, FOUT], FP32, name="zfin")
                nc.scalar.activation(zfin[:], zp2[:], AF.Relu)
                nc.sync.dma_start(z_out[:], zfin[:])

    nc.compile()
    nc.generate_event_semaphores()
    return nc


# ======================= public entry point =======================
_NC_CACHE = {}


def kernel(x, edge_index, batch, W1, b1, W2, b2, W3, b3, W4, b4,
           trace=False, upto=5):
    weights = dict(W1=np.asarray(W1, np.float32), b1=np.asarray(b1, np.float32),
                   W2=np.asarray(W2, np.float32), b2=np.asarray(b2, np.float32),
                   W3=np.asarray(W3, np.float32), b3=np.asarray(b3, np.float32),
                   W4=np.asarray(W4, np.float32), b4=np.asarray(b4, np.float32))
    n_cores = 8
    cfg, per_core, shared = build_plan(
        np.asarray(x, np.float32), np.asarray(edge_index), np.asarray(batch),
        weights, n_graphs=512, n_cores=n_cores)
    key = (upto, cfg["N"], cfg["NMAX"], cfg["n_win1"], cfg["n_win2"],
           cfg["T1"], cfg["T2"], cfg["n_idx16_1"], cfg["n_idx16_2"])
    if key not in _NC_CACHE:
        _NC_CACHE[key] = build_kernel(cfg, n_cores=n_cores, upto=upto)
    nc = _NC_CACHE[key]

    def bf16(a):
        import ml_dtypes
        return np.asarray(a).astype(ml_dtypes.bfloat16)

    base = dict(
        x_bf=bf16(shared["x_bf"]), w1aug=bf16(shared["w1aug"]),
        w2aug=bf16(shared["w2aug"]), w3aug=bf16(shared["w3aug"]),
        w4aug=bf16(shared["w4aug"]), b4row=bf16(shared["b4row"]),
        biases=shared["biases"],
    )
    maps = []
    for pc in per_core:
        m = dict(base)
        m["idx1"] = pc["idx1"]
        m["idx2"] = pc["idx2"]
        m["meta1"] = pc["meta1"]
        m["meta2"] = pc["meta2"]
        m["xtn"] = pc["xtn"]
        m["cnt1"] = pc["cnt1"]
        m["cnt2"] = pc["cnt2"]
        maps.append(m)
    res = run_bass_kernel_spmd(nc, maps, core_ids=list(range(n_cores)),
                               trace=trace)
    z = np.concatenate([res.results[c]["z"] for c in range(n_cores)], axis=0)
    kernel.last_results = res
    return z.astype(np.float32)


# revision 14
# speedup vs baseline: 2.2213x; 1.6340x over previous
"""Trainium2 Bass kernel for nn_DrugGCNncoder (2-layer GCN + max-pool + MLP).

Self-contained: accepts the FULL inputs of reference.setup_inputs(), shards
across 8 NeuronCores internally (dst-node/graph sharding), returns the FULL
[512, 128] output.

v2 design (vs v1 baseline):
 - bf16 gather tables, S-matrices and weights (2x DVE, half gather bytes).
 - W1 + relu + W2 fused per-node into the L1 window epilogue, producing
   z = relu(agg@W1+b1)@W2 directly; AllGather ships z (no dense phase 3,
   no transposes anywhere).
 - L1 self-loops folded into the epilogue via a host-precomputed
   norm_self * x^T tensor (removes them from the gather).
 - Window max-pool on the RAW aggregate; bias+relu applied after pooling
   (exact because relu is monotone; empty graphs correct because b2 == 0).
 - Graph-uniform window slots (Wmax per graph) -> compile-time segment
   reduce, no pooling masks.
 - Index padding with -1 sentinels: the gather ucode skips trailing -1s,
   so padded slots cost zero descriptors on each core.
 - 4 SWDGE queues, gather calls rotate across them.
 - AllGather split into 4 chunks, issued with one-group lag to overlap
   the transfer with the L1 tail.
"""
import sys
for p in ("/opt/trn_rl_repo", "/root/.axon_site/_ro/trn_rl_repo"):
    if p not in sys.path:
        sys.path.insert(0, p)
import numpy as np
import concourse.bass as bass
import concourse.bacc as bacc
import concourse.mybir as mybir
from concourse import tile
from concourse.bass_utils import run_bass_kernel_spmd

FP32 = mybir.dt.float32
BF16 = mybir.dt.bfloat16
I16 = mybir.dt.int16
AF = mybir.ActivationFunctionType
ALU = mybir.AluOpType

CHUNK_X = 32768      # x-table chunk rows (int16 index range)
DSTW = 256           # window width in dst-node columns
F1P = 128            # x padded feature count (bf16 -> 256B rows)
F2P = 384            # z padded feature count (bf16 -> 768B rows)
F1 = 78
F2 = 300
FOUT = 128
N_CORES = 8
N_GRAPHS = 512
GMAXI = 1024         # max rows per dma_gather call
NQ = 4               # SWDGE queues


def _pack_idx16(idx, cap):
    """idx (valid list) -> [128, cap//16] int16, slot j at [j%16, j//16],
    padded with -1 (skipped by the gather ucode), replicated 8x."""
    assert cap % 16 == 0 and len(idx) <= cap
    full = np.full(cap, -1, np.int16)
    full[: len(idx)] = idx
    blk = full.reshape(cap // 16, 16).T  # [16, cap/16]
    return np.tile(blk, (8, 1))  # [128, cap/16]


def build_plan(x, edge_index, batch, weights, n_graphs=512, n_cores=8):
    N = x.shape[0]
    G = n_graphs // n_cores
    src = edge_index[0].astype(np.int64)
    dst = edge_index[1].astype(np.int64)
    deg = (np.bincount(dst, minlength=N) + 1).astype(np.float64)  # + self loop
    dis = 1.0 / np.sqrt(deg)
    norm_e = (dis[src] * dis[dst]).astype(np.float32)
    norm_self = (dis * dis).astype(np.float32)

    batch = batch.astype(np.int64)
    g_start = np.searchsorted(batch, np.arange(n_graphs), side="left")
    g_end = np.searchsorted(batch, np.arange(n_graphs), side="right")
    node_start = [int(g_start[c * G]) for c in range(n_cores)]
    node_start.append(N)
    nodes_per_core = [node_start[c + 1] - node_start[c] for c in range(n_cores)]
    NMAX = ((max(nodes_per_core) + DSTW - 1) // DSTW) * DSTW
    n_win1 = NMAX // DSTW

    core_of = np.searchsorted(np.asarray(node_start[1:]), np.arange(N),
                              side="right")
    local_of = np.arange(N) - np.asarray(node_start)[core_of]

    # ---- z-table chunking (for chunked AllGather + int16 range) ----------
    ngrp = 4
    base_w = n_win1 // ngrp
    extra = n_win1 - base_w * ngrp
    grp_sizes = [base_w + (1 if j < extra else 0) for j in range(ngrp)]
    grp_w0 = np.cumsum([0] + grp_sizes)          # window offsets, len 5
    grp_rows = [s * DSTW for s in grp_sizes]     # local rows per group
    grp_r0 = np.cumsum([0] + grp_rows)           # local row offsets, len 5
    assert all(8 * r <= 32768 for r in grp_rows)
    # map local row -> (group j, row within z_full[j]) for a given core
    grp_of_local = np.searchsorted(grp_r0[1:], np.arange(NMAX), side="right")

    # ---- per-core dst-sorted edges --------------------------------------
    per_core_raw = []
    for c in range(n_cores):
        sel = (dst >= node_start[c]) & (dst < node_start[c + 1])
        s, d, nm = src[sel], dst[sel], norm_e[sel]
        dl = d - node_start[c]
        order = np.argsort(dl, kind="stable")
        per_core_raw.append((s[order], dl[order], nm[order]))

    # ---- L2 graph-uniform windows ---------------------------------------
    g_len = (g_end - g_start).astype(np.int64)
    Wmax = max(1, int((g_len.max() + DSTW - 1) // DSTW))
    n_win2 = G * Wmax

    # L1: source row in x table; chunk by global id // 32768
    n_chunks_x = (N + CHUNK_X - 1) // CHUNK_X
    # L2: source row in z_full[j]; j from the SOURCE node's local offset
    src_grp = grp_of_local[np.minimum(local_of, NMAX - 1)]
    src_zrow = (core_of * np.asarray(grp_rows)[src_grp]
                + (local_of - np.asarray(grp_r0)[src_grp]))
    n_chunks_z = ngrp

    def windows_l1(c):
        s_loc, dl, nm = per_core_raw[c]
        out = []
        for w in range(n_win1):
            lo = np.searchsorted(dl, w * DSTW, side="left")
            hi = np.searchsorted(dl, (w + 1) * DSTW, side="left")
            es, edl, enm = s_loc[lo:hi], dl[lo:hi] - w * DSTW, nm[lo:hi]
            ch = es // CHUNK_X
            runs = []
            for k in range(n_chunks_x):
                m = ch == k
                runs.append((es[m] - k * CHUNK_X, edl[m], enm[m]))
            out.append(runs)
        return out

    def windows_l2(c):
        s_loc, dl, nm = per_core_raw[c]
        # self-loop edges for this core's own nodes (kept in the L2 gather)
        own = np.arange(node_start[c], node_start[c + 1])
        sl_dl = own - node_start[c]
        all_src = np.concatenate([s_loc, own])
        all_dl = np.concatenate([dl, sl_dl])
        all_nm = np.concatenate([nm, norm_self[own]]).astype(np.float32)
        order = np.argsort(all_dl, kind="stable")
        all_src, all_dl, all_nm = all_src[order], all_dl[order], all_nm[order]
        out = []
        for gl in range(G):
            g = c * G + gl
            glo = int(g_start[g] - node_start[c])
            ghi = int(g_end[g] - node_start[c])
            for swin in range(Wmax):
                base = glo + swin * DSTW
                top = min(base + DSTW, ghi)
                lo = np.searchsorted(all_dl, base, side="left")
                hi = np.searchsorted(all_dl, max(top, base), side="left")
                es = all_src[lo:hi]
                edl = all_dl[lo:hi] - base
                enm = all_nm[lo:hi]
                rows = src_zrow[es]
                ch = src_grp[es] if len(es) else np.zeros(0, np.int64)
                runs = []
                for k in range(n_chunks_z):
                    m = ch == k
                    runs.append((rows[m], edl[m], enm[m]))
                out.append(runs)
        return out

    l1_cores = [windows_l1(c) for c in range(n_cores)]
    l2_cores = [windows_l2(c) for c in range(n_cores)]

    def normalize(cores_wins, n_win, n_chunks, force_first=False):
        caps = np.zeros((n_win, n_chunks), np.int64)
        for wins in cores_wins:
            for w in range(n_win):
                for k in range(n_chunks):
                    caps[w, k] = max(caps[w, k], len(wins[w][k][0]))
        caps = ((caps + 127) // 128) * 128
        if force_first:
            caps[:, 0] = np.maximum(caps[:, 0], 128)
        T = int(caps.sum(axis=1).max()) // 128
        return caps, T

    caps1, T1 = normalize(l1_cores, n_win1, n_chunks_x, force_first=True)
    caps2, T2 = normalize(l2_cores, n_win2, n_chunks_z, force_first=True)

    def emit(cores_wins, caps, n_win, T, n_chunks):
        n_idx16 = int(caps.sum()) // 16
        out = []
        for wins in cores_wins:
            idx16 = np.full((128, n_idx16), -1, np.int16)
            meta = np.zeros((n_win, 128, 2 * T), np.float32)
            meta[:, :, :T] = -1.0  # dstl pad
            col16 = 0
            for w in range(n_win):
                slot = 0
                for k in range(n_chunks):
                    cap = int(caps[w, k])
                    ri, rd, rn = wins[w][k]
                    idx16[:, col16 : col16 + cap // 16] = _pack_idx16(ri, cap)
                    n = len(ri)
                    sl = slot + np.arange(n)
                    meta[w, sl % 128, sl // 128] = rd.astype(np.float32)
                    meta[w, sl % 128, T + sl // 128] = rn
                    slot += cap
                    col16 += cap // 16
                assert slot <= T * 128
            out.append({"idx16": idx16,
                        "meta": meta.astype(np.float32)})  # cast to bf16 later
        return out

    l1_data = emit(l1_cores, caps1, n_win1, T1, n_chunks_x)
    l2_data = emit(l2_cores, caps2, n_win2, T2, n_chunks_z)

    def call_counts(cores_wins, caps, n_win, n_chunks):
        """Per-core valid-index count for every dma_gather call, in issue
        order (w, then k with cap>0, then GMAXI sub-calls)."""
        out = []
        for wins in cores_wins:
            cnts = []
            for w in range(n_win):
                for k in range(n_chunks):
                    cap = int(caps[w, k])
                    if cap == 0:
                        continue
                    nvalid = len(wins[w][k][0])
                    for off in range(0, cap, GMAXI):
                        sub = min(GMAXI, cap - off)
                        cnts.append(max(0, min(sub, nvalid - off)))
            out.append(np.asarray(cnts, np.int32).reshape(1, -1))
        return out

    cnt1 = call_counts(l1_cores, caps1, n_win1, n_chunks_x)
    cnt2 = call_counts(l2_cores, caps2, n_win2, n_chunks_z)

    def sched(caps):
        rows = []
        col16 = 0
        for w in range(caps.shape[0]):
            slot = 0
            ent = []
            for k in range(caps.shape[1]):
                cap = int(caps[w, k])
                if cap > 0:
                    ent.append((k, cap, slot, col16))
                slot += cap
                col16 += cap // 16
            rows.append((ent, slot))
        return rows

    # ---- packed weights (bf16) ------------------------------------------
    W1, b1, W2, b2, W3, b3, W4, b4 = (
        weights["W1"], weights["b1"], weights["W2"], weights["b2"],
        weights["W3"], weights["b3"], weights["W4"], weights["b4"],
    )
    w1aug = np.zeros((80, F2P), np.float32)
    w1aug[:F1, :F2] = W1
    w1aug[F1, :F2] = b1       # ones-row slot 78
    w2aug = np.zeros((F2P, F2P), np.float32)
    w2aug[:F2, :F2] = W2
    w3aug = np.zeros((F2P, 1024), np.float32)
    w3aug[:F2, :] = W3
    w4aug = np.zeros((1024, FOUT), np.float32)
    w4aug[:, :] = W4
    b4row = b4.reshape(1, FOUT).astype(np.float32)
    biases = np.zeros((128, 11), np.float32)
    for m in range(3):
        seg = np.zeros(128, np.float32)
        seg[: max(0, min(128, F2 - m * 128))] = b2[m * 128 : (m + 1) * 128]
        biases[:, m] = seg
    for m in range(8):
        biases[:, 3 + m] = b3[m * 128 : (m + 1) * 128]

    # x table bf16 [N, 128]
    x_bf = np.zeros((N, F1P), np.float32)
    x_bf[:, :F1] = x

    # per-core norm_self * x^T with ones row at 78
    xtn = []
    for c in range(n_cores):
        t = np.zeros((80, NMAX), np.float32)
        nn = nodes_per_core[c]
        own = np.arange(node_start[c], node_start[c + 1])
        t[:F1, :nn] = (x[own] * norm_self[own][:, None]).T
        t[F1, :] = 1.0
        xtn.append(t)

    cfg = dict(
        N=N, G=G, NMAX=NMAX, n_win1=n_win1, n_win2=n_win2, Wmax=Wmax,
        T1=T1, T2=T2, n_chunks_x=n_chunks_x, n_chunks_z=n_chunks_z,
        sched1=sched(caps1), sched2=sched(caps2),
        n_idx16_1=int(caps1.sum()) // 16, n_idx16_2=int(caps2.sum()) // 16,
        grp_sizes=grp_sizes, grp_rows=grp_rows,
        grp_w0=[int(v) for v in grp_w0], grp_r0=[int(v) for v in grp_r0],
        n_cores=n_cores, n_calls1=cnt1[0].shape[1], n_calls2=cnt2[0].shape[1],
    )
    shared = dict(x_bf=x_bf, w1aug=w1aug, w2aug=w2aug, w3aug=w3aug,
                  w4aug=w4aug, b4row=b4row, biases=biases)
    per_core = []
    for c in range(n_cores):
        per_core.append(dict(
            idx1=l1_data[c]["idx16"], meta1=l1_data[c]["meta"],
            idx2=l2_data[c]["idx16"], meta2=l2_data[c]["meta"],
            xtn=xtn[c], cnt1=cnt1[c], cnt2=cnt2[c],
        ))
    return cfg, per_core, shared


def build_kernel(cfg, n_cores=8, upto=5):
    G = cfg["G"]
    NMAX, n_win1, n_win2 = cfg["NMAX"], cfg["n_win1"], cfg["n_win2"]
    Wmax = cfg["Wmax"]
    T1, T2 = cfg["T1"], cfg["T2"]
    sched1, sched2 = cfg["sched1"], cfg["sched2"]
    grp_rows, grp_w0, grp_r0 = cfg["grp_rows"], cfg["grp_w0"], cfg["grp_r0"]
    ngrp = len(grp_rows)

    nc = bacc.Bacc("TRN2", target_bir_lowering=False, debug=False,
                   num_devices=n_cores, num_swdge_queues=NQ)

    # ---- I/O ----
    x_bf = nc.dram_tensor("x_bf", [cfg["N"], F1P], BF16, kind="ExternalInput")
    xtn_in = nc.dram_tensor("xtn", [80, NMAX], FP32, kind="ExternalInput")
    idx1 = nc.dram_tensor("idx1", [128, cfg["n_idx16_1"]], I16,
                          kind="ExternalInput")
    idx2 = nc.dram_tensor("idx2", [128, cfg["n_idx16_2"]], I16,
                          kind="ExternalInput")
    meta1 = nc.dram_tensor("meta1", [n_win1, 128, 2 * T1], FP32,
                           kind="ExternalInput")
    meta2 = nc.dram_tensor("meta2", [n_win2, 128, 2 * T2], FP32,
                           kind="ExternalInput")
    w1_in = nc.dram_tensor("w1aug", [80, F2P], BF16, kind="ExternalInput")
    w2_in = nc.dram_tensor("w2aug", [F2P, F2P], BF16, kind="ExternalInput")
    w3_in = nc.dram_tensor("w3aug", [F2P, 1024], BF16, kind="ExternalInput")
    w4_in = nc.dram_tensor("w4aug", [1024, FOUT], BF16, kind="ExternalInput")
    b4_in = nc.dram_tensor("b4row", [1, FOUT], BF16, kind="ExternalInput")
    bias_in = nc.dram_tensor("biases", [128, 11], FP32, kind="ExternalInput")
    cnt1_in = nc.dram_tensor("cnt1", [1, cfg["n_calls1"]], mybir.dt.int32,
                             kind="ExternalInput")
    cnt2_in = nc.dram_tensor("cnt2", [1, cfg["n_calls2"]], mybir.dt.int32,
                             kind="ExternalInput")
    z_out = nc.dram_tensor("z", [G, FOUT], FP32, kind="ExternalOutput")
    if upto == 1:
        dbg1 = nc.dram_tensor("dbg1", [NMAX, F2P], BF16, kind="ExternalOutput")
    if upto == 2:
        dbg2 = nc.dram_tensor("dbg2", [8 * grp_rows[0], F2P], BF16,
                              kind="ExternalOutput")

    with tile.TileContext(nc) as tc, \
         tc.tile_pool(name="dram", bufs=1, space="DRAM") as drp, \
         tc.tile_pool(name="consts", bufs=1) as consts:
        z_me = drp.tile([NMAX, F2P], BF16, name="z_me")
        z_full = [drp.tile([n_cores * grp_rows[j], F2P], BF16,
                           addr_space="Shared", name=f"z_full{j}")
                  for j in range(ngrp)]

        iota_i32 = consts.tile([128, DSTW], mybir.dt.int32)
        nc.gpsimd.iota(iota_i32[:], [[1, DSTW]], base=0, channel_multiplier=0)
        iota_bf = consts.tile([128, DSTW], BF16)
        nc.vector.tensor_copy(iota_bf[:], iota_i32[:])
        w1_sb = consts.tile([80, F2P], BF16)
        nc.sync.dma_start(w1_sb[:], w1_in[:])
        w2_sb = [consts.tile([128, F2P], BF16, name=f"w2_{k}") for k in range(3)]
        for k in range(3):
            nc.sync.dma_start(w2_sb[k][:], w2_in[k * 128 : (k + 1) * 128, :])
        w3_sb = [consts.tile([128, 1024], BF16, name=f"w3_{k}") for k in range(3)]
        for k in range(3):
            nc.sync.dma_start(w3_sb[k][:], w3_in[k * 128 : (k + 1) * 128, :])
        w4_sb = [consts.tile([128, FOUT], BF16, name=f"w4_{k}") for k in range(8)]
        for k in range(8):
            nc.sync.dma_start(w4_sb[k][:], w4_in[k * 128 : (k + 1) * 128, :])
        b4_sb = consts.tile([1, FOUT], BF16)
        nc.sync.dma_start(b4_sb[:], b4_in[:])
        bias_sb = consts.tile([128, 11], FP32)
        nc.sync.dma_start(bias_sb[:], bias_in[:])
        ones64 = consts.tile([1, G], BF16)
        nc.vector.memset(ones64[:], 1.0)
        cnt1_sb = consts.tile([1, cfg["n_calls1"]], mybir.dt.int32)
        nc.sync.dma_start(cnt1_sb[:], cnt1_in[:])
        cnt2_sb = consts.tile([1, cfg["n_calls2"]], mybir.dt.int32)
        nc.sync.dma_start(cnt2_sb[:], cnt2_in[:])
        cnt_regs = [nc.gpsimd.alloc_register(f"cnt_reg{i}") for i in range(4)]
        pooled_win = [consts.tile([128, G, Wmax], FP32, name=f"pw{m}")
                      for m in range(3)]

        qc = [0]

        def gather_window(gpool, ipool, w, sched, idx_hbm, tables, T, F, tag,
                          memset_first, cnt_sb, call_i):
            ent, tot = sched[w]
            gbuf = gpool.tile([128, T, F], BF16, tag="gbuf",
                              name=f"gbuf_{tag}_{w}", padded_shape=[128, T, F])
            if memset_first:
                nc.vector.memset(gbuf[:], 0.0)
            c16_0 = ent[0][3]
            c16_n = ent[-1][3] + ent[-1][1] // 16
            itile = ipool.tile([128, c16_n - c16_0], I16, tag="idx",
                               name=f"idx_{tag}_{w}")
            nc.sync.dma_start(itile[:], idx_hbm[:, c16_0:c16_n])
            for (k, cap, slot, c16) in ent:
                table = tables[k]
                for off in range(0, cap, GMAXI):
                    sub = min(GMAXI, cap - off)
                    so = slot + off
                    co = c16 - c16_0 + off // 16
                    ci = call_i[0]
                    call_i[0] += 1
                    nval = cnt_regs[ci % 4]
                    nc.gpsimd.reg_load(nval, cnt_sb[0:1, ci : ci + 1])
                    nc.gpsimd.dma_gather(
                        gbuf[:, so // 128 : (so + sub) // 128, :],
                        table,
                        itile[:, co : co + sub // 16],
                        sub, nval, F,
                        queue_num=qc[0] % NQ,
                    )
                    qc[0] += 1
            return gbuf, tot // 128

        # =============== Phase 1: L1 windows + fused node transform =======
        x_tables = [x_bf[k * CHUNK_X : min((k + 1) * CHUNK_X, cfg["N"]), :]
                    for k in range(cfg["n_chunks_x"])]
        with tc.tile_pool(name="gp1", bufs=2) as gpool, \
             tc.tile_pool(name="ip1", bufs=3) as ipool, \
             tc.tile_pool(name="mp1", bufs=2) as mpool, \
             tc.tile_pool(name="sp1", bufs=4) as spool, \
             tc.tile_pool(name="sb1", bufs=3) as sbp, \
             tc.tile_pool(name="ps_agg", bufs=2, space="PSUM") as psA, \
             tc.tile_pool(name="ps_h1", bufs=2, space="PSUM") as psB, \
             tc.tile_pool(name="ps_z", bufs=2, space="PSUM") as psC:
            pending_cc = []
            call1 = [0]
            for j in range(ngrp):
                for w in range(grp_w0[j], grp_w0[j + 1]):
                    gbuf, nt = gather_window(gpool, ipool, w, sched1, idx1,
                                             x_tables, T1, F1P, "l1", w < 2,
                                             cnt1_sb, call1)
                    meta = mpool.tile([128, 2 * T1], FP32, tag="meta",
                                      name=f"m1_{w}")
                    nc.sync.dma_start(meta[:], meta1[w])
                    xw = mpool.tile([80, DSTW], FP32, tag="xtn",
                                    name=f"xw_{w}")
                    nc.sync.dma_start(
                        xw[:], xtn_in[:, w * DSTW : (w + 1) * DSTW])
                    agg = psA.tile([80, DSTW], FP32, tag="agg",
                                   name=f"agg_{w}")
                    for t in range(nt):
                        S = spool.tile([128, DSTW], BF16, tag="S",
                                       name=f"S1_{w}_{t}")
                        nc.vector.tensor_scalar(
                            S[:], iota_bf[:], meta[:, t : t + 1],
                            meta[:, T1 + t : T1 + t + 1],
                            ALU.is_equal, ALU.mult)
                        nc.tensor.matmul(agg[:], gbuf[:, t, 0:80], S[:],
                                         start=(t == 0), stop=(t == nt - 1))
                    asb = sbp.tile([80, DSTW], BF16, tag="asb",
                                   name=f"asb_{w}")
                    nc.vector.tensor_tensor(asb[:], agg[:], xw[:], ALU.add)
                    zp = [psC.tile([128, F2P], FP32, tag=f"zp{h}",
                                   name=f"zp_{w}_{h}") for h in range(2)]
                    for ki in range(3):
                        hp = psB.tile([128, DSTW], FP32, tag="hp",
                                      name=f"hp_{w}_{ki}")
                        nc.tensor.matmul(
                            hp[:], w1_sb[0:79, ki * 128 : (ki + 1) * 128],
                            asb[0:79, :], start=True, stop=True)
                        ht = sbp.tile([128, DSTW], BF16, tag="ht",
                                      name=f"ht_{w}_{ki}")
                        nc.scalar.activation(ht[:], hp[:], AF.Relu)
                        for h in range(2):
                            nc.tensor.matmul(
                                zp[h][:], ht[:, h * 128 : (h + 1) * 128],
                                w2_sb[ki][:], start=(ki == 0), stop=(ki == 2))
                    for h in range(2):
                        zsb = sbp.tile([128, F2P], BF16, tag="zsb",
                                       name=f"zsb_{w}_{h}")
                        nc.scalar.activation(zsb[:], zp[h][:], AF.Copy)
                        nc.sync.dma_start(
                            z_me[w * DSTW + h * 128 : w * DSTW + (h + 1) * 128,
                                 :], zsb[:])
                # lag-one-group collective issue to avoid stalling gathers
                pending_cc.append(j)
                if upto >= 2 and len(pending_cc) > 1:
                    jj = pending_cc.pop(0)
                    nc.gpsimd.collective_compute(
                        "AllGather", ALU.bypass,
                        replica_groups=[list(range(n_cores))],
                        ins=[z_me[grp_r0[jj] : grp_r0[jj + 1], :].opt()],
                        outs=[z_full[jj][:].opt()],
                    )
            for jj in (pending_cc if upto >= 2 else []):
                nc.gpsimd.collective_compute(
                    "AllGather", ALU.bypass,
                    replica_groups=[list(range(n_cores))],
                    ins=[z_me[grp_r0[jj] : grp_r0[jj + 1], :].opt()],
                    outs=[z_full[jj][:].opt()],
                )

        if upto == 1:
            nc.sync.dma_start(dbg1[:], z_me[:])
        if upto == 2:
            nc.sync.dma_start(dbg2[:], z_full[0][:])

        # =============== Phase 2: L2 windows + raw-agg pooling =============
        z_tables = [z_full[k][:] for k in range(ngrp)]
        with tc.tile_pool(name="gp2", bufs=2) as gpool, \
             tc.tile_pool(name="ip2", bufs=3) as ipool, \
             tc.tile_pool(name="mp2", bufs=2) as mpool, \
             tc.tile_pool(name="sp2", bufs=4) as spool, \
             tc.tile_pool(name="ps_a2", bufs=2, space="PSUM") as ps2:
            call2 = [0]
            for w in range(n_win2 if upto >= 4 else 0):
                gbuf, nt = gather_window(gpool, ipool, w, sched2, idx2,
                                         z_tables, T2, F2P, "l2", w < 2,
                                         cnt2_sb, call2)
                meta = mpool.tile([128, 2 * T2], FP32, tag="meta",
                                  name=f"m2_{w}")
                nc.sync.dma_start(meta[:], meta2[w])
                aggs = [ps2.tile([128, DSTW], FP32, tag=f"a2_{fi}",
                                 name=f"a2_{w}_{fi}") for fi in range(3)]
                for t in range(nt):
                    S = spool.tile([128, DSTW], BF16, tag="S",
                                   name=f"S2_{w}_{t}")
                    nc.vector.tensor_scalar(
                        S[:], iota_bf[:], meta[:, t : t + 1],
                        meta[:, T2 + t : T2 + t + 1], ALU.is_equal, ALU.mult)
                    for fi in range(3):
                        nc.tensor.matmul(
                            aggs[fi][:], gbuf[:, t, fi * 128 : (fi + 1) * 128],
                            S[:], start=(t == 0), stop=(t == nt - 1))
                gl, sw = w // Wmax, w % Wmax
                for fi in range(3):
                    nc.vector.tensor_reduce(
                        pooled_win[fi][:, gl, sw : sw + 1], aggs[fi][:],
                        axis=mybir.AxisListType.X, op=ALU.max)

        # =============== Phase 3: pool combine + MLP =======================
        if upto >= 4:
            with tc.tile_pool(name="p5", bufs=2) as p5, \
                 tc.tile_pool(name="ps_mlp", bufs=4, space="PSUM") as psz, \
                 tc.tile_pool(name="zsb5", bufs=1) as zsbp:
                pooledTr = []
                for m in range(3):
                    praw = p5.tile([128, G], FP32, tag="praw",
                                   name=f"praw{m}")
                    nc.vector.tensor_reduce(
                        praw[:], pooled_win[m][:],
                        axis=mybir.AxisListType.X, op=ALU.max)
                    pr = zsbp.tile([128, G], BF16, name=f"pTr{m}")
                    nc.scalar.activation(pr[:], praw[:], AF.Relu,
                                         bias=bias_sb[:, m : m + 1])
                    pooledTr.append(pr)
                z1t = []
                for mi in range(8):
                    zp = psz.tile([128, G], FP32, tag="z1p",
                                  name=f"z1p_{mi}")
                    for ki in range(3):
                        nc.tensor.matmul(
                            zp[:],
                            w3_sb[ki][:, mi * 128 : (mi + 1) * 128],
                            pooledTr[ki][:], start=(ki == 0), stop=(ki == 2))
                    zt = zsbp.tile([128, G], BF16, name=f"z1t_{mi}")
                    nc.scalar.activation(zt[:], zp[:], AF.Relu,
                                         bias=bias_sb[:, 3 + mi : 4 + mi])
                    z1t.append(zt)
                zp2 = psz.tile([G, FOUT], FP32, tag="z2p", name="z2p")
                for ki in range(9):
                    lhsT = z1t[ki][:] if ki < 8 else ones64[:]
                    rhs = w4_sb[ki][:] if ki < 8 else b4_sb[:]
                    nc.tensor.matmul(zp2[:], lhsT, rhs,
                                     start=(ki == 0), stop=(ki == 8))
                zfin = zsbp.tile([G, FOUT], FP32, name="zfin")
                nc.scalar.activation(zfin[:], zp2[:], AF.Relu)
                nc.sync.dma_start(z_out[:], zfin[:])

    nc.compile()
    nc.generate_event_semaphores()
    return nc


# ======================= public entry point =======================
_NC_CACHE = {}


def kernel(x, edge_index, batch, W1, b1, W2, b2, W3, b3, W4, b4,
           trace=False, upto=5):
    weights = dict(W1=np.asarray(W1, np.float32), b1=np.asarray(b1, np.float32),
                   W2=np.asarray(W2, np.float32), b2=np.asarray(b2, np.float32),
                   W3=np.asarray(W3, np.float32), b3=np.asarray(b3, np.float32),
                   W4=np.asarray(W4, np.float32), b4=np.asarray(b4, np.float32))
    n_cores = 8
    cfg, per_core, shared = build_plan(
        np.asarray(x, np.float32), np.asarray(edge_index), np.asarray(batch),
        weights, n_graphs=512, n_cores=n_cores)
    key = (upto, cfg["N"], cfg["NMAX"], cfg["n_win1"], cfg["n_win2"],
           cfg["T1"], cfg["T2"], cfg["n_idx16_1"], cfg["n_idx16_2"])
    if key not in _NC_CACHE:
        _NC_CACHE[key] = build_kernel(cfg, n_cores=n_cores, upto=upto)
    nc = _NC_CACHE[key]

    def bf16(a):
        import ml_dtypes
        return np.asarray(a).astype(ml_dtypes.bfloat16)

    base = dict(
        x_bf=bf16(shared["x_bf"]), w1aug=bf16(shared["w1aug"]),
        w2aug=bf16(shared["w2aug"]), w3aug=bf16(shared["w3aug"]),
        w4aug=bf16(shared["w4aug"]), b4row=bf16(shared["b4row"]),
        biases=shared["biases"],
    )
    maps = []
    for pc in per_core:
        m = dict(base)
        m["idx1"] = pc["idx1"]
        m["idx2"] = pc["idx2"]
        m["meta1"] = pc["meta1"]
        m["meta2"] = pc["meta2"]
        m["xtn"] = pc["xtn"]
        m["cnt1"] = pc["cnt1"]
        m["cnt2"] = pc["cnt2"]
        maps.append(m)
    res = run_bass_kernel_spmd(nc, maps, core_ids=list(range(n_cores)),
                               trace=trace)
    z = np.concatenate([res.results[c]["z"] for c in range(n_cores)], axis=0)
    kernel.last_results = res
    return z.astype(np.float32)


# revision 16
# speedup vs baseline: 2.2222x; 1.0004x over previous
"""Trainium2 Bass kernel for nn_DrugGCNncoder (2-layer GCN + max-pool + MLP).

Self-contained: accepts the FULL inputs of reference.setup_inputs(), shards
across 8 NeuronCores internally (dst-node/graph sharding), returns the FULL
[512, 128] output.

v2 design (vs v1 baseline):
 - bf16 gather tables, S-matrices and weights (2x DVE, half gather bytes).
 - W1 + relu + W2 fused per-node into the L1 window epilogue, producing
   z = relu(agg@W1+b1)@W2 directly; AllGather ships z (no dense phase 3,
   no transposes anywhere).
 - L1 self-loops folded into the epilogue via a host-precomputed
   norm_self * x^T tensor (removes them from the gather).
 - Window max-pool on the RAW aggregate; bias+relu applied after pooling
   (exact because relu is monotone; empty graphs correct because b2 == 0).
 - Graph-uniform window slots (Wmax per graph) -> compile-time segment
   reduce, no pooling masks.
 - Index padding with -1 sentinels: the gather ucode skips trailing -1s,
   so padded slots cost zero descriptors on each core.
 - 4 SWDGE queues, gather calls rotate across them.
 - AllGather split into 4 chunks, issued with one-group lag to overlap
   the transfer with the L1 tail.
"""
import sys
for p in ("/opt/trn_rl_repo", "/root/.axon_site/_ro/trn_rl_repo"):
    if p not in sys.path:
        sys.path.insert(0, p)
import numpy as np
import concourse.bass as bass
import concourse.bacc as bacc
import concourse.mybir as mybir
from concourse import tile
from concourse.bass_utils import run_bass_kernel_spmd

FP32 = mybir.dt.float32
BF16 = mybir.dt.bfloat16
I16 = mybir.dt.int16
AF = mybir.ActivationFunctionType
ALU = mybir.AluOpType

CHUNK_X = 32768      # x-table chunk rows (int16 index range)
DSTW = 256           # window width in dst-node columns
F1P = 128            # x padded feature count (bf16 -> 256B rows)
F2P = 384            # z padded feature count (bf16 -> 768B rows)
F1 = 78
F2 = 300
FOUT = 128
N_CORES = 8
N_GRAPHS = 512
GMAXI = 1024         # max rows per dma_gather call
NQ = 4               # SWDGE queues


def _pack_idx16(idx, cap):
    """idx (valid list) -> [128, cap//16] int16, slot j at [j%16, j//16],
    padded with -1 (skipped by the gather ucode), replicated 8x."""
    assert cap % 16 == 0 and len(idx) <= cap
    full = np.full(cap, -1, np.int16)
    full[: len(idx)] = idx
    blk = full.reshape(cap // 16, 16).T  # [16, cap/16]
    return np.tile(blk, (8, 1))  # [128, cap/16]


def build_plan(x, edge_index, batch, weights, n_graphs=512, n_cores=8):
    N = x.shape[0]
    G = n_graphs // n_cores
    src = edge_index[0].astype(np.int64)
    dst = edge_index[1].astype(np.int64)
    deg = (np.bincount(dst, minlength=N) + 1).astype(np.float64)  # + self loop
    dis = 1.0 / np.sqrt(deg)
    norm_e = (dis[src] * dis[dst]).astype(np.float32)
    norm_self = (dis * dis).astype(np.float32)

    batch = batch.astype(np.int64)
    g_start = np.searchsorted(batch, np.arange(n_graphs), side="left")
    g_end = np.searchsorted(batch, np.arange(n_graphs), side="right")
    node_start = [int(g_start[c * G]) for c in range(n_cores)]
    node_start.append(N)
    nodes_per_core = [node_start[c + 1] - node_start[c] for c in range(n_cores)]
    NMAX = ((max(nodes_per_core) + DSTW - 1) // DSTW) * DSTW
    n_win1 = NMAX // DSTW

    core_of = np.searchsorted(np.asarray(node_start[1:]), np.arange(N),
                              side="right")
    local_of = np.arange(N) - np.asarray(node_start)[core_of]

    # ---- z-table chunking (for chunked AllGather + int16 range) ----------
    ngrp = 4
    base_w = n_win1 // ngrp
    extra = n_win1 - base_w * ngrp
    grp_sizes = [base_w + (1 if j < extra else 0) for j in range(ngrp)]
    grp_w0 = np.cumsum([0] + grp_sizes)          # window offsets, len 5
    grp_rows = [s * DSTW for s in grp_sizes]     # local rows per group
    grp_r0 = np.cumsum([0] + grp_rows)           # local row offsets, len 5
    assert all(8 * r <= 32768 for r in grp_rows)
    # map local row -> (group j, row within z_full[j]) for a given core
    grp_of_local = np.searchsorted(grp_r0[1:], np.arange(NMAX), side="right")

    # ---- per-core dst-sorted edges --------------------------------------
    per_core_raw = []
    for c in range(n_cores):
        sel = (dst >= node_start[c]) & (dst < node_start[c + 1])
        s, d, nm = src[sel], dst[sel], norm_e[sel]
        dl = d - node_start[c]
        order = np.argsort(dl, kind="stable")
        per_core_raw.append((s[order], dl[order], nm[order]))

    # ---- L2 graph-uniform windows ---------------------------------------
    g_len = (g_end - g_start).astype(np.int64)
    Wmax = max(1, int((g_len.max() + DSTW - 1) // DSTW))
    n_win2 = G * Wmax

    # L1: source row in x table; chunk by global id // 32768
    n_chunks_x = (N + CHUNK_X - 1) // CHUNK_X
    # L2: source row in z_full[j]; j from the SOURCE node's local offset
    src_grp = grp_of_local[np.minimum(local_of, NMAX - 1)]
    src_zrow = (core_of * np.asarray(grp_rows)[src_grp]
                + (local_of - np.asarray(grp_r0)[src_grp]))
    n_chunks_z = ngrp

    def windows_l1(c):
        s_loc, dl, nm = per_core_raw[c]
        out = []
        for w in range(n_win1):
            lo = np.searchsorted(dl, w * DSTW, side="left")
            hi = np.searchsorted(dl, (w + 1) * DSTW, side="left")
            es, edl, enm = s_loc[lo:hi], dl[lo:hi] - w * DSTW, nm[lo:hi]
            ch = es // CHUNK_X
            runs = []
            for k in range(n_chunks_x):
                m = ch == k
                runs.append((es[m] - k * CHUNK_X, edl[m], enm[m]))
            out.append(runs)
        return out

    def windows_l2(c):
        s_loc, dl, nm = per_core_raw[c]
        # self-loop edges for this core's own nodes (kept in the L2 gather)
        own = np.arange(node_start[c], node_start[c + 1])
        sl_dl = own - node_start[c]
        all_src = np.concatenate([s_loc, own])
        all_dl = np.concatenate([dl, sl_dl])
        all_nm = np.concatenate([nm, norm_self[own]]).astype(np.float32)
        order = np.argsort(all_dl, kind="stable")
        all_src, all_dl, all_nm = all_src[order], all_dl[order], all_nm[order]
        out = []
        for gl in range(G):
            g = c * G + gl
            glo = int(g_start[g] - node_start[c])
            ghi = int(g_end[g] - node_start[c])
            for swin in range(Wmax):
                base = glo + swin * DSTW
                top = min(base + DSTW, ghi)
                lo = np.searchsorted(all_dl, base, side="left")
                hi = np.searchsorted(all_dl, max(top, base), side="left")
                es = all_src[lo:hi]
                edl = all_dl[lo:hi] - base
                enm = all_nm[lo:hi]
                rows = src_zrow[es]
                ch = src_grp[es] if len(es) else np.zeros(0, np.int64)
                runs = []
                for k in range(n_chunks_z):
                    m = ch == k
                    runs.append((rows[m], edl[m], enm[m]))
                out.append(runs)
        return out

    l1_cores = [windows_l1(c) for c in range(n_cores)]
    l2_cores = [windows_l2(c) for c in range(n_cores)]

    def normalize(cores_wins, n_win, n_chunks, force_first=False):
        caps = np.zeros((n_win, n_chunks), np.int64)
        for wins in cores_wins:
            for w in range(n_win):
                for k in range(n_chunks):
                    caps[w, k] = max(caps[w, k], len(wins[w][k][0]))
        caps = ((caps + 127) // 128) * 128
        if force_first:
            caps[:, 0] = np.maximum(caps[:, 0], 128)
        T = int(caps.sum(axis=1).max()) // 128
        return caps, T

    caps1, T1 = normalize(l1_cores, n_win1, n_chunks_x, force_first=True)
    caps2, T2 = normalize(l2_cores, n_win2, n_chunks_z, force_first=True)

    def emit(cores_wins, caps, n_win, T, n_chunks):
        n_idx16 = int(caps.sum()) // 16
        out = []
        for wins in cores_wins:
            idx16 = np.full((128, n_idx16), -1, np.int16)
            meta = np.zeros((n_win, 128, 4 * T), np.float32)
            meta[:, :, :T] = -1.0       # dstl pad
            meta[:, :, 2 * T : 3 * T] = 1.0  # -dstl pad
            col16 = 0
            for w in range(n_win):
                slot = 0
                for k in range(n_chunks):
                    cap = int(caps[w, k])
                    ri, rd, rn = wins[w][k]
                    idx16[:, col16 : col16 + cap // 16] = _pack_idx16(ri, cap)
                    n = len(ri)
                    sl = slot + np.arange(n)
                    rdf = rd.astype(np.float32)
                    meta[w, sl % 128, sl // 128] = rdf
                    meta[w, sl % 128, T + sl // 128] = rn
                    meta[w, sl % 128, 2 * T + sl // 128] = -rdf
                    meta[w, sl % 128, 3 * T + sl // 128] = -rn
                    slot += cap
                    col16 += cap // 16
                assert slot <= T * 128
            out.append({"idx16": idx16,
                        "meta": meta.astype(np.float32)})  # cast to bf16 later
        return out

    l1_data = emit(l1_cores, caps1, n_win1, T1, n_chunks_x)
    l2_data = emit(l2_cores, caps2, n_win2, T2, n_chunks_z)

    def call_counts(cores_wins, caps, n_win, n_chunks):
        """Per-core valid-index count for every dma_gather call, in issue
        order (w, then k with cap>0, then GMAXI sub-calls)."""
        out = []
        for wins in cores_wins:
            cnts = []
            for w in range(n_win):
                for k in range(n_chunks):
                    cap = int(caps[w, k])
                    if cap == 0:
                        continue
                    nvalid = len(wins[w][k][0])
                    for off in range(0, cap, GMAXI):
                        sub = min(GMAXI, cap - off)
                        cnts.append(max(0, min(sub, nvalid - off)))
            out.append(np.asarray(cnts, np.int32).reshape(1, -1))
        return out

    cnt1 = call_counts(l1_cores, caps1, n_win1, n_chunks_x)
    cnt2 = call_counts(l2_cores, caps2, n_win2, n_chunks_z)

    def sched(caps):
        rows = []
        col16 = 0
        for w in range(caps.shape[0]):
            slot = 0
            ent = []
            for k in range(caps.shape[1]):
                cap = int(caps[w, k])
                if cap > 0:
                    ent.append((k, cap, slot, col16))
                slot += cap
                col16 += cap // 16
            rows.append((ent, slot))
        return rows

    # ---- packed weights (bf16) ------------------------------------------
    W1, b1, W2, b2, W3, b3, W4, b4 = (
        weights["W1"], weights["b1"], weights["W2"], weights["b2"],
        weights["W3"], weights["b3"], weights["W4"], weights["b4"],
    )
    w1aug = np.zeros((80, F2P), np.float32)
    w1aug[:F1, :F2] = W1
    w1aug[F1, :F2] = b1       # ones-row slot 78
    w2aug = np.zeros((F2P, F2P), np.float32)
    w2aug[:F2, :F2] = W2
    w3aug = np.zeros((F2P, 1024), np.float32)
    w3aug[:F2, :] = W3
    w4aug = np.zeros((1024, FOUT), np.float32)
    w4aug[:, :] = W4
    b4row = b4.reshape(1, FOUT).astype(np.float32)
    biases = np.zeros((128, 11), np.float32)
    for m in range(3):
        seg = np.zeros(128, np.float32)
        seg[: max(0, min(128, F2 - m * 128))] = b2[m * 128 : (m + 1) * 128]
        biases[:, m] = seg
    for m in range(8):
        biases[:, 3 + m] = b3[m * 128 : (m + 1) * 128]

    # x table bf16 [N, 128]
    x_bf = np.zeros((N, F1P), np.float32)
    x_bf[:, :F1] = x

    # per-core norm_self * x^T with ones row at 78
    xtn = []
    for c in range(n_cores):
        t = np.zeros((80, NMAX), np.float32)
        nn = nodes_per_core[c]
        own = np.arange(node_start[c], node_start[c + 1])
        t[:F1, :nn] = (x[own] * norm_self[own][:, None]).T
        t[F1, :] = 1.0
        xtn.append(t)

    cfg = dict(
        N=N, G=G, NMAX=NMAX, n_win1=n_win1, n_win2=n_win2, Wmax=Wmax,
        T1=T1, T2=T2, n_chunks_x=n_chunks_x, n_chunks_z=n_chunks_z,
        sched1=sched(caps1), sched2=sched(caps2),
        n_idx16_1=int(caps1.sum()) // 16, n_idx16_2=int(caps2.sum()) // 16,
        grp_sizes=grp_sizes, grp_rows=grp_rows,
        grp_w0=[int(v) for v in grp_w0], grp_r0=[int(v) for v in grp_r0],
        n_cores=n_cores, n_calls1=cnt1[0].shape[1], n_calls2=cnt2[0].shape[1],
    )
    shared = dict(x_bf=x_bf, w1aug=w1aug, w2aug=w2aug, w3aug=w3aug,
                  w4aug=w4aug, b4row=b4row, biases=biases)
    per_core = []
    for c in range(n_cores):
        per_core.append(dict(
            idx1=l1_data[c]["idx16"], meta1=l1_data[c]["meta"],
            idx2=l2_data[c]["idx16"], meta2=l2_data[c]["meta"],
            xtn=xtn[c], cnt1=cnt1[c], cnt2=cnt2[c],
        ))
    return cfg, per_core, shared


def build_kernel(cfg, n_cores=8, upto=5):
    G = cfg["G"]
    NMAX, n_win1, n_win2 = cfg["NMAX"], cfg["n_win1"], cfg["n_win2"]
    Wmax = cfg["Wmax"]
    T1, T2 = cfg["T1"], cfg["T2"]
    sched1, sched2 = cfg["sched1"], cfg["sched2"]
    grp_rows, grp_w0, grp_r0 = cfg["grp_rows"], cfg["grp_w0"], cfg["grp_r0"]
    ngrp = len(grp_rows)

    nc = bacc.Bacc("TRN2", target_bir_lowering=False, debug=False,
                   num_devices=n_cores, num_swdge_queues=NQ)

    # ---- I/O ----
    x_bf = nc.dram_tensor("x_bf", [cfg["N"], F1P], BF16, kind="ExternalInput")
    xtn_in = nc.dram_tensor("xtn", [80, NMAX], FP32, kind="ExternalInput")
    idx1 = nc.dram_tensor("idx1", [128, cfg["n_idx16_1"]], I16,
                          kind="ExternalInput")
    idx2 = nc.dram_tensor("idx2", [128, cfg["n_idx16_2"]], I16,
                          kind="ExternalInput")
    meta1 = nc.dram_tensor("meta1", [n_win1, 128, 4 * T1], FP32,
                           kind="ExternalInput")
    meta2 = nc.dram_tensor("meta2", [n_win2, 128, 4 * T2], FP32,
                           kind="ExternalInput")
    w1_in = nc.dram_tensor("w1aug", [80, F2P], BF16, kind="ExternalInput")
    w2_in = nc.dram_tensor("w2aug", [F2P, F2P], BF16, kind="ExternalInput")
    w3_in = nc.dram_tensor("w3aug", [F2P, 1024], BF16, kind="ExternalInput")
    w4_in = nc.dram_tensor("w4aug", [1024, FOUT], BF16, kind="ExternalInput")
    b4_in = nc.dram_tensor("b4row", [1, FOUT], BF16, kind="ExternalInput")
    bias_in = nc.dram_tensor("biases", [128, 11], FP32, kind="ExternalInput")
    cnt1_in = nc.dram_tensor("cnt1", [1, cfg["n_calls1"]], mybir.dt.int32,
                             kind="ExternalInput")
    cnt2_in = nc.dram_tensor("cnt2", [1, cfg["n_calls2"]], mybir.dt.int32,
                             kind="ExternalInput")
    z_out = nc.dram_tensor("z", [G, FOUT], FP32, kind="ExternalOutput")
    if upto == 1:
        dbg1 = nc.dram_tensor("dbg1", [NMAX, F2P], BF16, kind="ExternalOutput")
    if upto == 2:
        dbg2 = nc.dram_tensor("dbg2", [8 * grp_rows[0], F2P], BF16,
                              kind="ExternalOutput")

    with tile.TileContext(nc) as tc, \
         tc.tile_pool(name="dram", bufs=1, space="DRAM") as drp, \
         tc.tile_pool(name="consts", bufs=1) as consts:
        z_me = drp.tile([NMAX, F2P], BF16, name="z_me")
        z_full = [drp.tile([n_cores * grp_rows[j], F2P], BF16,
                           addr_space="Shared", name=f"z_full{j}")
                  for j in range(ngrp)]

        iota_i32 = consts.tile([128, DSTW], mybir.dt.int32)
        nc.gpsimd.iota(iota_i32[:], [[1, DSTW]], base=0, channel_multiplier=0)
        iota_bf = consts.tile([128, DSTW], BF16)
        nc.vector.tensor_copy(iota_bf[:], iota_i32[:])
        w1_sb = consts.tile([80, F2P], BF16)
        nc.sync.dma_start(w1_sb[:], w1_in[:])
        w2_sb = [consts.tile([128, F2P], BF16, name=f"w2_{k}") for k in range(3)]
        for k in range(3):
            nc.sync.dma_start(w2_sb[k][:], w2_in[k * 128 : (k + 1) * 128, :])
        w3_sb = [consts.tile([128, 1024], BF16, name=f"w3_{k}") for k in range(3)]
        for k in range(3):
            nc.sync.dma_start(w3_sb[k][:], w3_in[k * 128 : (k + 1) * 128, :])
        w4_sb = [consts.tile([128, FOUT], BF16, name=f"w4_{k}") for k in range(8)]
        for k in range(8):
            nc.sync.dma_start(w4_sb[k][:], w4_in[k * 128 : (k + 1) * 128, :])
        b4_sb = consts.tile([1, FOUT], BF16)
        nc.sync.dma_start(b4_sb[:], b4_in[:])
        bias_sb = consts.tile([128, 11], FP32)
        nc.sync.dma_start(bias_sb[:], bias_in[:])
        ones64 = consts.tile([1, G], BF16)
        nc.vector.memset(ones64[:], 1.0)
        cnt1_sb = consts.tile([1, cfg["n_calls1"]], mybir.dt.int32)
        nc.sync.dma_start(cnt1_sb[:], cnt1_in[:])
        cnt2_sb = consts.tile([1, cfg["n_calls2"]], mybir.dt.int32)
        nc.sync.dma_start(cnt2_sb[:], cnt2_in[:])
        cnt_regs = [nc.gpsimd.alloc_register(f"cnt_reg{i}") for i in range(4)]
        pooled_win = [consts.tile([128, G, Wmax], FP32, name=f"pw{m}")
                      for m in range(3)]

        qc = [0]

        s_ctr = [0]

        def build_S(spool, tpool, w, t, meta, T, tag):
            """S[p, j] = norm_p * 1[iota_j == dstl_p], routed to DVE or ACT."""
            S = spool.tile([128, DSTW], BF16, tag="S", name=f"S_{tag}_{w}_{t}")
            i = s_ctr[0]
            s_ctr[0] += 1
            if i % 5 < 3:
                nc.vector.tensor_scalar(
                    S[:], iota_bf[:], meta[:, t : t + 1],
                    meta[:, T + t : T + t + 1], ALU.is_equal, ALU.mult)
            else:
                sq = tpool.tile([128, DSTW], BF16, tag="sq",
                                name=f"sq_{tag}_{w}_{t}")
                nc.scalar.activation(sq[:], iota_bf[:], AF.Square,
                                     bias=meta[:, 2 * T + t : 2 * T + t + 1])
                nc.scalar.activation(S[:], sq[:], AF.Relu,
                                     scale=meta[:, 3 * T + t : 3 * T + t + 1],
                                     bias=meta[:, T + t : T + t + 1])
            return S

        def gather_window(gpool, ipool, w, sched, idx_hbm, tables, T, F, tag,
                          memset_first, cnt_sb, call_i):
            ent, tot = sched[w]
            gbuf = gpool.tile([128, T, F], BF16, tag="gbuf",
                              name=f"gbuf_{tag}_{w}", padded_shape=[128, T, F])
            if memset_first:
                nc.vector.memset(gbuf[:], 0.0)
            c16_0 = ent[0][3]
            c16_n = ent[-1][3] + ent[-1][1] // 16
            itile = ipool.tile([128, c16_n - c16_0], I16, tag="idx",
                               name=f"idx_{tag}_{w}")
            nc.sync.dma_start(itile[:], idx_hbm[:, c16_0:c16_n])
            for (k, cap, slot, c16) in ent:
                table = tables[k]
                for off in range(0, cap, GMAXI):
                    sub = min(GMAXI, cap - off)
                    so = slot + off
                    co = c16 - c16_0 + off // 16
                    ci = call_i[0]
                    call_i[0] += 1
                    nval = cnt_regs[ci % 4]
                    nc.gpsimd.reg_load(nval, cnt_sb[0:1, ci : ci + 1])
                    nc.gpsimd.dma_gather(
                        gbuf[:, so // 128 : (so + sub) // 128, :],
                        table,
                        itile[:, co : co + sub // 16],
                        sub, nval, F,
                        queue_num=qc[0] % NQ,
                    )
                    qc[0] += 1
            return gbuf, tot // 128

        # =============== Phase 1: L1 windows + fused node transform =======
        x_tables = [x_bf[k * CHUNK_X : min((k + 1) * CHUNK_X, cfg["N"]), :]
                    for k in range(cfg["n_chunks_x"])]
        with tc.tile_pool(name="gp1", bufs=2) as gpool, \
             tc.tile_pool(name="ip1", bufs=3) as ipool, \
             tc.tile_pool(name="mp1", bufs=2) as mpool, \
             tc.tile_pool(name="sp1", bufs=4) as spool, \
             tc.tile_pool(name="sb1", bufs=3) as sbp, \
             tc.tile_pool(name="ps_agg", bufs=2, space="PSUM") as psA, \
             tc.tile_pool(name="ps_h1", bufs=2, space="PSUM") as psB, \
             tc.tile_pool(name="ps_z", bufs=2, space="PSUM") as psC:
            pending_cc = []
            call1 = [0]
            for j in range(ngrp):
                for w in range(grp_w0[j], grp_w0[j + 1]):
                    gbuf, nt = gather_window(gpool, ipool, w, sched1, idx1,
                                             x_tables, T1, F1P, "l1", w < 2,
                                             cnt1_sb, call1)
                    meta = mpool.tile([128, 4 * T1], FP32, tag="meta",
                                      name=f"m1_{w}")
                    nc.sync.dma_start(meta[:], meta1[w])
                    xw = mpool.tile([80, DSTW], FP32, tag="xtn",
                                    name=f"xw_{w}")
                    nc.sync.dma_start(
                        xw[:], xtn_in[:, w * DSTW : (w + 1) * DSTW])
                    agg = psA.tile([80, DSTW], FP32, tag="agg",
                                   name=f"agg_{w}")
                    for t in range(nt):
                        S = build_S(spool, spool, w, t, meta, T1, "l1")
                        nc.tensor.matmul(agg[:], gbuf[:, t, 0:80], S[:],
                                         start=(t == 0), stop=(t == nt - 1))
                    asb = sbp.tile([80, DSTW], BF16, tag="asb",
                                   name=f"asb_{w}")
                    nc.vector.tensor_tensor(asb[:], agg[:], xw[:], ALU.add)
                    zp = [psC.tile([128, F2P], FP32, tag=f"zp{h}",
                                   name=f"zp_{w}_{h}") for h in range(2)]
                    for ki in range(3):
                        hp = psB.tile([128, DSTW], FP32, tag="hp",
                                      name=f"hp_{w}_{ki}")
                        nc.tensor.matmul(
                            hp[:], w1_sb[0:79, ki * 128 : (ki + 1) * 128],
                            asb[0:79, :], start=True, stop=True)
                        ht = sbp.tile([128, DSTW], BF16, tag="ht",
                                      name=f"ht_{w}_{ki}")
                        nc.scalar.activation(ht[:], hp[:], AF.Relu)
                        for h in range(2):
                            nc.tensor.matmul(
                                zp[h][:], ht[:, h * 128 : (h + 1) * 128],
                                w2_sb[ki][:], start=(ki == 0), stop=(ki == 2))
                    for h in range(2):
                        zsb = sbp.tile([128, F2P], BF16, tag="zsb",
                                       name=f"zsb_{w}_{h}")
                        nc.scalar.activation(zsb[:], zp[h][:], AF.Copy)
                        nc.sync.dma_start(
                            z_me[w * DSTW + h * 128 : w * DSTW + (h + 1) * 128,
                                 :], zsb[:])
                # lag-one-group collective issue to avoid stalling gathers
                pending_cc.append(j)
                if upto >= 2 and len(pending_cc) > 1:
                    jj = pending_cc.pop(0)
                    nc.gpsimd.collective_compute(
                        "AllGather", ALU.bypass,
                        replica_groups=[list(range(n_cores))],
                        ins=[z_me[grp_r0[jj] : grp_r0[jj + 1], :].opt()],
                        outs=[z_full[jj][:].opt()],
                    )
            for jj in (pending_cc if upto >= 2 else []):
                nc.gpsimd.collective_compute(
                    "AllGather", ALU.bypass,
                    replica_groups=[list(range(n_cores))],
                    ins=[z_me[grp_r0[jj] : grp_r0[jj + 1], :].opt()],
                    outs=[z_full[jj][:].opt()],
                )

        if upto == 1:
            nc.sync.dma_start(dbg1[:], z_me[:])
        if upto == 2:
            nc.sync.dma_start(dbg2[:], z_full[0][:])

        # =============== Phase 2: L2 windows + raw-agg pooling =============
        z_tables = [z_full[k][:] for k in range(ngrp)]
        with tc.tile_pool(name="gp2", bufs=2) as gpool, \
             tc.tile_pool(name="ip2", bufs=3) as ipool, \
             tc.tile_pool(name="mp2", bufs=2) as mpool, \
             tc.tile_pool(name="sp2", bufs=4) as spool, \
             tc.tile_pool(name="ps_a2", bufs=2, space="PSUM") as ps2:
            call2 = [0]
            for w in range(n_win2 if upto >= 4 else 0):
                gbuf, nt = gather_window(gpool, ipool, w, sched2, idx2,
                                         z_tables, T2, F2P, "l2", w < 2,
                                         cnt2_sb, call2)
                meta = mpool.tile([128, 4 * T2], FP32, tag="meta",
                                  name=f"m2_{w}")
                nc.sync.dma_start(meta[:], meta2[w])
                aggs = [ps2.tile([128, DSTW], FP32, tag=f"a2_{fi}",
                                 name=f"a2_{w}_{fi}") for fi in range(3)]
                for t in range(nt):
                    S = build_S(spool, spool, w, t, meta, T2, "l2")
                    for fi in range(3):
                        nc.tensor.matmul(
                            aggs[fi][:], gbuf[:, t, fi * 128 : (fi + 1) * 128],
                            S[:], start=(t == 0), stop=(t == nt - 1))
                gl, sw = w // Wmax, w % Wmax
                for fi in range(3):
                    nc.vector.tensor_reduce(
                        pooled_win[fi][:, gl, sw : sw + 1], aggs[fi][:],
                        axis=mybir.AxisListType.X, op=ALU.max)

        # =============== Phase 3: pool combine + MLP =======================
        if upto >= 4:
            with tc.tile_pool(name="p5", bufs=2) as p5, \
                 tc.tile_pool(name="ps_mlp", bufs=4, space="PSUM") as psz, \
                 tc.tile_pool(name="zsb5", bufs=1) as zsbp:
                pooledTr = []
                for m in range(3):
                    praw = p5.tile([128, G], FP32, tag="praw",
                                   name=f"praw{m}")
                    nc.vector.tensor_reduce(
                        praw[:], pooled_win[m][:],
                        axis=mybir.AxisListType.X, op=ALU.max)
                    pr = zsbp.tile([128, G], BF16, name=f"pTr{m}")
                    nc.scalar.activation(pr[:], praw[:], AF.Relu,
                                         bias=bias_sb[:, m : m + 1])
                    pooledTr.append(pr)
                z1t = []
                for mi in range(8):
                    zp = psz.tile([128, G], FP32, tag="z1p",
                                  name=f"z1p_{mi}")
                    for ki in range(3):
                        nc.tensor.matmul(
                            zp[:],
                            w3_sb[ki][:, mi * 128 : (mi + 1) * 128],
                            pooledTr[ki][:], start=(ki == 0), stop=(ki == 2))
                    zt = zsbp.tile([128, G], BF16, name=f"z1t_{mi}")
                    nc.scalar.activation(zt[:], zp[:], AF.Relu,
                                         bias=bias_sb[:, 3 + mi : 4 + mi])
                    z1t.append(zt)
                zp2 = psz.tile([G, FOUT], FP32, tag="z2p", name="z2p")
                for ki in range(9):
                    lhsT = z1t[ki][:] if ki < 8 else ones64[:]
                    rhs = w4_sb[ki][:] if ki < 8 else b4_sb[:]
                    nc.tensor.matmul(zp2[:], lhsT, rhs,
                                     start=(ki == 0), stop=(ki == 8))
                zfin = zsbp.tile([G, FOUT], FP32, name="zfin")
                nc.scalar.activation(zfin[:], zp2[:], AF.Relu)
                nc.sync.dma_start(z_out[:], zfin[:])

    nc.compile()
    nc.generate_event_semaphores()
    return nc


# ======================= public entry point =======================
_NC_CACHE = {}


def kernel(x, edge_index, batch, W1, b1, W2, b2, W3, b3, W4, b4,
           trace=False, upto=5):
    weights = dict(W1=np.asarray(W1, np.float32), b1=np.asarray(b1, np.float32),
                   W2=np.asarray(W2, np.float32), b2=np.asarray(b2, np.float32),
                   W3=np.asarray(W3, np.float32), b3=np.asarray(b3, np.float32),
                   W4=np.asarray(W4, np.float32), b4=np.asarray(b4, np.float32))
    n_cores = 8
    cfg, per_core, shared = build_plan(
        np.asarray(x, np.float32), np.asarray(edge_index), np.asarray(batch),
        weights, n_graphs=512, n_cores=n_cores)
    key = (upto, cfg["N"], cfg["NMAX"], cfg["n_win1"], cfg["n_win2"],
           cfg["T1"], cfg["T2"], cfg["n_idx16_1"], cfg["n_idx16_2"])
    if key not in _NC_CACHE:
        _NC_CACHE[key] = build_kernel(cfg, n_cores=n_cores, upto=upto)
    nc = _NC_CACHE[key]

    def bf16(a):
        import ml_dtypes
        return np.asarray(a).astype(ml_dtypes.bfloat16)

    base = dict(
        x_bf=bf16(shared["x_bf"]), w1aug=bf16(shared["w1aug"]),
        w2aug=bf16(shared["w2aug"]), w3aug=bf16(shared["w3aug"]),
        w4aug=bf16(shared["w4aug"]), b4row=bf16(shared["b4row"]),
        biases=shared["biases"],
    )
    maps = []
    for pc in per_core:
        m = dict(base)
        m["idx1"] = pc["idx1"]
        m["idx2"] = pc["idx2"]
        m["meta1"] = pc["meta1"]
        m["meta2"] = pc["meta2"]
        m["xtn"] = pc["xtn"]
        m["cnt1"] = pc["cnt1"]
        m["cnt2"] = pc["cnt2"]
        maps.append(m)
    res = run_bass_kernel_spmd(nc, maps, core_ids=list(range(n_cores)),
                               trace=trace)
    z = np.concatenate([res.results[c]["z"] for c in range(n_cores)], axis=0)
    kernel.last_results = res
    return z.astype(np.float32)


# revision 18
# speedup vs baseline: 2.5182x; 1.1332x over previous
"""Trainium2 Bass kernel for nn_DrugGCNncoder (2-layer GCN + max-pool + MLP).

Self-contained: accepts the FULL inputs of reference.setup_inputs(), shards
across 8 NeuronCores internally (dst-node/graph sharding), returns the FULL
[512, 128] output.

v2 design (vs v1 baseline):
 - bf16 gather tables, S-matrices and weights (2x DVE, half gather bytes).
 - W1 + relu + W2 fused per-node into the L1 window epilogue, producing
   z = relu(agg@W1+b1)@W2 directly; AllGather ships z (no dense phase 3,
   no transposes anywhere).
 - L1 self-loops folded into the epilogue via a host-precomputed
   norm_self * x^T tensor (removes them from the gather).
 - Window max-pool on the RAW aggregate; bias+relu applied after pooling
   (exact because relu is monotone; empty graphs correct because b2 == 0).
 - Graph-uniform window slots (Wmax per graph) -> compile-time segment
   reduce, no pooling masks.
 - Index padding with -1 sentinels: the gather ucode skips trailing -1s,
   so padded slots cost zero descriptors on each core.
 - 4 SWDGE queues, gather calls rotate across them.
 - AllGather split into 4 chunks, issued with one-group lag to overlap
   the transfer with the L1 tail.
"""
import sys
for p in ("/opt/trn_rl_repo", "/root/.axon_site/_ro/trn_rl_repo"):
    if p not in sys.path:
        sys.path.insert(0, p)
import numpy as np
import concourse.bass as bass
import concourse.bacc as bacc
import concourse.mybir as mybir
from concourse import tile
from concourse.bass_utils import run_bass_kernel_spmd

FP32 = mybir.dt.float32
BF16 = mybir.dt.bfloat16
I16 = mybir.dt.int16
AF = mybir.ActivationFunctionType
ALU = mybir.AluOpType

CHUNK_X = 32768      # x-table chunk rows (int16 index range)
DSTW = 256           # window width in dst-node columns
F1P = 128            # x padded feature count (bf16 -> 256B rows)
F2P = 384            # z padded feature count (bf16 -> 768B rows)
F1 = 78
F2 = 300
FOUT = 128
N_CORES = 8
N_GRAPHS = 512
GMAXI = 1024         # max rows per dma_gather call
NQ = 4               # SWDGE queues


def _pack_idx16(idx, cap):
    """idx (valid list) -> [128, cap//16] int16, slot j at [j%16, j//16],
    padded with -1 (skipped by the gather ucode), replicated 8x."""
    assert cap % 16 == 0 and len(idx) <= cap
    full = np.full(cap, -1, np.int16)
    full[: len(idx)] = idx
    blk = full.reshape(cap // 16, 16).T  # [16, cap/16]
    return np.tile(blk, (8, 1))  # [128, cap/16]


def build_plan(x, edge_index, batch, weights, n_graphs=512, n_cores=8):
    N = x.shape[0]
    G = n_graphs // n_cores
    src = edge_index[0].astype(np.int64)
    dst = edge_index[1].astype(np.int64)
    deg = (np.bincount(dst, minlength=N) + 1).astype(np.float64)  # + self loop
    dis = 1.0 / np.sqrt(deg)
    norm_e = (dis[src] * dis[dst]).astype(np.float32)
    norm_self = (dis * dis).astype(np.float32)

    batch = batch.astype(np.int64)
    g_start = np.searchsorted(batch, np.arange(n_graphs), side="left")
    g_end = np.searchsorted(batch, np.arange(n_graphs), side="right")
    node_start = [int(g_start[c * G]) for c in range(n_cores)]
    node_start.append(N)
    nodes_per_core = [node_start[c + 1] - node_start[c] for c in range(n_cores)]
    NMAX = ((max(nodes_per_core) + DSTW - 1) // DSTW) * DSTW
    n_win1 = NMAX // DSTW

    core_of = np.searchsorted(np.asarray(node_start[1:]), np.arange(N),
                              side="right")
    local_of = np.arange(N) - np.asarray(node_start)[core_of]

    # ---- z-table chunking (for chunked AllGather + int16 range) ----------
    ngrp = 4
    base_w = n_win1 // ngrp
    extra = n_win1 - base_w * ngrp
    grp_sizes = [base_w + (1 if j < extra else 0) for j in range(ngrp)]
    grp_w0 = np.cumsum([0] + grp_sizes)          # window offsets, len 5
    grp_rows = [s * DSTW for s in grp_sizes]     # local rows per group
    grp_r0 = np.cumsum([0] + grp_rows)           # local row offsets, len 5
    assert all(8 * r <= 32768 for r in grp_rows)
    # map local row -> (group j, row within z_full[j]) for a given core
    grp_of_local = np.searchsorted(grp_r0[1:], np.arange(NMAX), side="right")

    # ---- per-core dst-sorted edges --------------------------------------
    per_core_raw = []
    for c in range(n_cores):
        sel = (dst >= node_start[c]) & (dst < node_start[c + 1])
        s, d, nm = src[sel], dst[sel], norm_e[sel]
        dl = d - node_start[c]
        order = np.argsort(dl, kind="stable")
        per_core_raw.append((s[order], dl[order], nm[order]))

    # ---- L2 graph-uniform windows ---------------------------------------
    g_len = (g_end - g_start).astype(np.int64)
    Wmax = max(1, int((g_len.max() + DSTW - 1) // DSTW))
    n_win2 = G * Wmax

    # L1: source row in x table; chunk by global id // 32768
    n_chunks_x = (N + CHUNK_X - 1) // CHUNK_X
    # L2: source row in z_full[j]; j from the SOURCE node's local offset
    src_grp = grp_of_local[np.minimum(local_of, NMAX - 1)]
    src_zrow = (core_of * np.asarray(grp_rows)[src_grp]
                + (local_of - np.asarray(grp_r0)[src_grp]))
    n_chunks_z = ngrp

    def windows_l1(c):
        s_loc, dl, nm = per_core_raw[c]
        out = []
        for w in range(n_win1):
            lo = np.searchsorted(dl, w * DSTW, side="left")
            hi = np.searchsorted(dl, (w + 1) * DSTW, side="left")
            es, edl, enm = s_loc[lo:hi], dl[lo:hi] - w * DSTW, nm[lo:hi]
            ch = es // CHUNK_X
            runs = []
            for k in range(n_chunks_x):
                m = ch == k
                runs.append((es[m] - k * CHUNK_X, edl[m], enm[m]))
            out.append(runs)
        return out

    def windows_l2(c):
        s_loc, dl, nm = per_core_raw[c]
        # self-loop edges for this core's own nodes (kept in the L2 gather)
        own = np.arange(node_start[c], node_start[c + 1])
        sl_dl = own - node_start[c]
        all_src = np.concatenate([s_loc, own])
        all_dl = np.concatenate([dl, sl_dl])
        all_nm = np.concatenate([nm, norm_self[own]]).astype(np.float32)
        order = np.argsort(all_dl, kind="stable")
        all_src, all_dl, all_nm = all_src[order], all_dl[order], all_nm[order]
        out = []
        for gl in range(G):
            g = c * G + gl
            glo = int(g_start[g] - node_start[c])
            ghi = int(g_end[g] - node_start[c])
            for swin in range(Wmax):
                base = glo + swin * DSTW
                top = min(base + DSTW, ghi)
                lo = np.searchsorted(all_dl, base, side="left")
                hi = np.searchsorted(all_dl, max(top, base), side="left")
                es = all_src[lo:hi]
                edl = all_dl[lo:hi] - base
                enm = all_nm[lo:hi]
                rows = src_zrow[es]
                ch = src_grp[es] if len(es) else np.zeros(0, np.int64)
                runs = []
                for k in range(n_chunks_z):
                    m = ch == k
                    runs.append((rows[m], edl[m], enm[m]))
                out.append(runs)
        return out

    l1_cores = [windows_l1(c) for c in range(n_cores)]
    l2_cores = [windows_l2(c) for c in range(n_cores)]

    def normalize(cores_wins, n_win, n_chunks, force_first=False):
        caps = np.zeros((n_win, n_chunks), np.int64)
        for wins in cores_wins:
            for w in range(n_win):
                for k in range(n_chunks):
                    caps[w, k] = max(caps[w, k], len(wins[w][k][0]))
        caps = ((caps + 127) // 128) * 128
        if force_first:
            caps[:, 0] = np.maximum(caps[:, 0], 128)
        T = int(caps.sum(axis=1).max()) // 128
        return caps, T

    caps1, T1 = normalize(l1_cores, n_win1, n_chunks_x, force_first=True)
    caps2, T2 = normalize(l2_cores, n_win2, n_chunks_z, force_first=True)

    def emit(cores_wins, caps, n_win, T, n_chunks):
        n_idx16 = int(caps.sum()) // 16
        out = []
        for wins in cores_wins:
            idx16 = np.full((128, n_idx16), -1, np.int16)
            meta = np.zeros((n_win, 128, 4 * T), np.float32)
            meta[:, :, :T] = -1.0       # dstl pad
            meta[:, :, 2 * T : 3 * T] = 1.0  # -dstl pad
            col16 = 0
            for w in range(n_win):
                slot = 0
                for k in range(n_chunks):
                    cap = int(caps[w, k])
                    ri, rd, rn = wins[w][k]
                    idx16[:, col16 : col16 + cap // 16] = _pack_idx16(ri, cap)
                    n = len(ri)
                    sl = slot + np.arange(n)
                    rdf = rd.astype(np.float32)
                    meta[w, sl % 128, sl // 128] = rdf
                    meta[w, sl % 128, T + sl // 128] = rn
                    meta[w, sl % 128, 2 * T + sl // 128] = -rdf
                    meta[w, sl % 128, 3 * T + sl // 128] = -rn
                    slot += cap
                    col16 += cap // 16
                assert slot <= T * 128
            out.append({"idx16": idx16,
                        "meta": meta.astype(np.float32)})  # cast to bf16 later
        return out

    l1_data = emit(l1_cores, caps1, n_win1, T1, n_chunks_x)
    l2_data = emit(l2_cores, caps2, n_win2, T2, n_chunks_z)

    def call_counts(cores_wins, caps, n_win, n_chunks):
        """Per-core valid-index count for every dma_gather call, in issue
        order (w, then k with cap>0, then GMAXI sub-calls)."""
        out = []
        for wins in cores_wins:
            cnts = []
            for w in range(n_win):
                for k in range(n_chunks):
                    cap = int(caps[w, k])
                    if cap == 0:
                        continue
                    nvalid = len(wins[w][k][0])
                    for off in range(0, cap, GMAXI):
                        sub = min(GMAXI, cap - off)
                        cnts.append(max(0, min(sub, nvalid - off)))
            out.append(np.asarray(cnts, np.int32).reshape(1, -1))
        return out

    cnt1 = call_counts(l1_cores, caps1, n_win1, n_chunks_x)
    cnt2 = call_counts(l2_cores, caps2, n_win2, n_chunks_z)

    def sched(caps):
        rows = []
        col16 = 0
        for w in range(caps.shape[0]):
            slot = 0
            ent = []
            for k in range(caps.shape[1]):
                cap = int(caps[w, k])
                if cap > 0:
                    ent.append((k, cap, slot, col16))
                slot += cap
                col16 += cap // 16
            rows.append((ent, slot))
        return rows

    # ---- packed weights (bf16) ------------------------------------------
    W1, b1, W2, b2, W3, b3, W4, b4 = (
        weights["W1"], weights["b1"], weights["W2"], weights["b2"],
        weights["W3"], weights["b3"], weights["W4"], weights["b4"],
    )
    w1aug = np.zeros((80, F2P), np.float32)
    w1aug[:F1, :F2] = W1
    w1aug[F1, :F2] = b1       # ones-row slot 78
    w2aug = np.zeros((F2P, F2P), np.float32)
    w2aug[:F2, :F2] = W2
    w3aug = np.zeros((F2P, 1024), np.float32)
    w3aug[:F2, :] = W3
    w4aug = np.zeros((1024, FOUT), np.float32)
    w4aug[:, :] = W4
    b4row = b4.reshape(1, FOUT).astype(np.float32)
    biases = np.zeros((128, 11), np.float32)
    for m in range(3):
        seg = np.zeros(128, np.float32)
        seg[: max(0, min(128, F2 - m * 128))] = b2[m * 128 : (m + 1) * 128]
        biases[:, m] = seg
    for m in range(8):
        biases[:, 3 + m] = b3[m * 128 : (m + 1) * 128]

    # x table bf16 [N, 128]
    x_bf = np.zeros((N, F1P), np.float32)
    x_bf[:, :F1] = x

    # per-core norm_self * x^T with ones row at 78
    xtn = []
    for c in range(n_cores):
        t = np.zeros((80, NMAX), np.float32)
        nn = nodes_per_core[c]
        own = np.arange(node_start[c], node_start[c + 1])
        t[:F1, :nn] = (x[own] * norm_self[own][:, None]).T
        t[F1, :] = 1.0
        xtn.append(t)

    cfg = dict(
        N=N, G=G, NMAX=NMAX, n_win1=n_win1, n_win2=n_win2, Wmax=Wmax,
        T1=T1, T2=T2, n_chunks_x=n_chunks_x, n_chunks_z=n_chunks_z,
        sched1=sched(caps1), sched2=sched(caps2),
        n_idx16_1=int(caps1.sum()) // 16, n_idx16_2=int(caps2.sum()) // 16,
        grp_sizes=grp_sizes, grp_rows=grp_rows,
        grp_w0=[int(v) for v in grp_w0], grp_r0=[int(v) for v in grp_r0],
        n_cores=n_cores, n_calls1=cnt1[0].shape[1], n_calls2=cnt2[0].shape[1],
    )
    shared = dict(x_bf=x_bf, w1aug=w1aug, w2aug=w2aug, w3aug=w3aug,
                  w4aug=w4aug, b4row=b4row, biases=biases)
    per_core = []
    for c in range(n_cores):
        per_core.append(dict(
            idx1=l1_data[c]["idx16"], meta1=l1_data[c]["meta"],
            idx2=l2_data[c]["idx16"], meta2=l2_data[c]["meta"],
            xtn=xtn[c], cnt1=cnt1[c], cnt2=cnt2[c],
        ))
    return cfg, per_core, shared


def build_kernel(cfg, n_cores=8, upto=5):
    G = cfg["G"]
    NMAX, n_win1, n_win2 = cfg["NMAX"], cfg["n_win1"], cfg["n_win2"]
    Wmax = cfg["Wmax"]
    T1, T2 = cfg["T1"], cfg["T2"]
    sched1, sched2 = cfg["sched1"], cfg["sched2"]
    grp_rows, grp_w0, grp_r0 = cfg["grp_rows"], cfg["grp_w0"], cfg["grp_r0"]
    ngrp = len(grp_rows)

    nc = bacc.Bacc("TRN2", target_bir_lowering=False, debug=False,
                   num_devices=n_cores, num_swdge_queues=NQ)

    # ---- I/O ----
    x_bf = nc.dram_tensor("x_bf", [cfg["N"], F1P], BF16, kind="ExternalInput")
    xtn_in = nc.dram_tensor("xtn", [80, NMAX], FP32, kind="ExternalInput")
    idx1 = nc.dram_tensor("idx1", [128, cfg["n_idx16_1"]], I16,
                          kind="ExternalInput")
    idx2 = nc.dram_tensor("idx2", [128, cfg["n_idx16_2"]], I16,
                          kind="ExternalInput")
    meta1 = nc.dram_tensor("meta1", [n_win1, 128, 4 * T1], FP32,
                           kind="ExternalInput")
    meta2 = nc.dram_tensor("meta2", [n_win2, 128, 4 * T2], FP32,
                           kind="ExternalInput")
    w1_in = nc.dram_tensor("w1aug", [80, F2P], BF16, kind="ExternalInput")
    w2_in = nc.dram_tensor("w2aug", [F2P, F2P], BF16, kind="ExternalInput")
    w3_in = nc.dram_tensor("w3aug", [F2P, 1024], BF16, kind="ExternalInput")
    w4_in = nc.dram_tensor("w4aug", [1024, FOUT], BF16, kind="ExternalInput")
    b4_in = nc.dram_tensor("b4row", [1, FOUT], BF16, kind="ExternalInput")
    bias_in = nc.dram_tensor("biases", [128, 11], FP32, kind="ExternalInput")
    cnt1_in = nc.dram_tensor("cnt1", [1, cfg["n_calls1"]], mybir.dt.int32,
                             kind="ExternalInput")
    cnt2_in = nc.dram_tensor("cnt2", [1, cfg["n_calls2"]], mybir.dt.int32,
                             kind="ExternalInput")
    z_out = nc.dram_tensor("z", [G, FOUT], FP32, kind="ExternalOutput")
    if upto == 1:
        dbg1 = nc.dram_tensor("dbg1", [NMAX, F2P], BF16, kind="ExternalOutput")
    if upto == 2:
        dbg2 = nc.dram_tensor("dbg2", [8 * grp_rows[0], F2P], BF16,
                              kind="ExternalOutput")

    with tile.TileContext(nc) as tc, \
         tc.tile_pool(name="dram", bufs=1, space="DRAM") as drp, \
         tc.tile_pool(name="consts", bufs=1) as consts:
        z_me = drp.tile([NMAX, F2P], BF16, name="z_me")
        z_full = [drp.tile([n_cores * grp_rows[j], F2P], BF16,
                           addr_space="Shared", name=f"z_full{j}")
                  for j in range(ngrp)]

        iota_i32 = consts.tile([128, DSTW], mybir.dt.int32)
        nc.gpsimd.iota(iota_i32[:], [[1, DSTW]], base=0, channel_multiplier=0)
        iota_bf = consts.tile([128, DSTW], BF16)
        nc.vector.tensor_copy(iota_bf[:], iota_i32[:])
        w1_sb = consts.tile([80, F2P], BF16)
        nc.sync.dma_start(w1_sb[:], w1_in[:])
        w2_sb = [consts.tile([128, F2P], BF16, name=f"w2_{k}") for k in range(3)]
        for k in range(3):
            nc.sync.dma_start(w2_sb[k][:], w2_in[k * 128 : (k + 1) * 128, :])
        w3_sb = [consts.tile([128, 1024], BF16, name=f"w3_{k}") for k in range(3)]
        for k in range(3):
            nc.sync.dma_start(w3_sb[k][:], w3_in[k * 128 : (k + 1) * 128, :])
        w4_sb = [consts.tile([128, FOUT], BF16, name=f"w4_{k}") for k in range(8)]
        for k in range(8):
            nc.sync.dma_start(w4_sb[k][:], w4_in[k * 128 : (k + 1) * 128, :])
        b4_sb = consts.tile([1, FOUT], BF16)
        nc.sync.dma_start(b4_sb[:], b4_in[:])
        bias_sb = consts.tile([128, 11], FP32)
        nc.sync.dma_start(bias_sb[:], bias_in[:])
        ones64 = consts.tile([1, G], BF16)
        nc.vector.memset(ones64[:], 1.0)
        cnt1_sb = consts.tile([1, cfg["n_calls1"]], mybir.dt.int32)
        nc.sync.dma_start(cnt1_sb[:], cnt1_in[:])
        cnt2_sb = consts.tile([1, cfg["n_calls2"]], mybir.dt.int32)
        nc.sync.dma_start(cnt2_sb[:], cnt2_in[:])
        cnt_regs = [nc.gpsimd.alloc_register(f"cnt_reg{i}") for i in range(4)]
        pooled_win = [consts.tile([128, G, Wmax], FP32, name=f"pw{m}")
                      for m in range(3)]

        qc = [0]

        s_ctr = [0]

        def build_S(spool, tpool, w, t, meta, T, tag):
            """S[p, j] = norm_p * 1[iota_j == dstl_p], routed to DVE or ACT."""
            S = spool.tile([128, DSTW], BF16, tag="S", name=f"S_{tag}_{w}_{t}")
            i = s_ctr[0]
            s_ctr[0] += 1
            if i % 5 < 4:
                nc.vector.tensor_scalar(
                    S[:], iota_bf[:], meta[:, t : t + 1],
                    meta[:, T + t : T + t + 1], ALU.is_equal, ALU.mult)
            else:
                sq = tpool.tile([128, DSTW], BF16, tag="sq",
                                name=f"sq_{tag}_{w}_{t}")
                nc.scalar.activation(sq[:], iota_bf[:], AF.Square,
                                     bias=meta[:, 2 * T + t : 2 * T + t + 1])
                nc.scalar.activation(S[:], sq[:], AF.Relu,
                                     scale=meta[:, 3 * T + t : 3 * T + t + 1],
                                     bias=meta[:, T + t : T + t + 1])
            return S

        def gather_window(gpool, ipool, w, sched, idx_hbm, tables, T, F, tag,
                          memset_first, cnt_sb, call_i):
            ent, tot = sched[w]
            gbuf = gpool.tile([128, T, F], BF16, tag="gbuf",
                              name=f"gbuf_{tag}_{w}", padded_shape=[128, T, F])
            if memset_first:
                nc.vector.memset(gbuf[:], 0.0)
            c16_0 = ent[0][3]
            c16_n = ent[-1][3] + ent[-1][1] // 16
            itile = ipool.tile([128, c16_n - c16_0], I16, tag="idx",
                               name=f"idx_{tag}_{w}")
            nc.sync.dma_start(itile[:], idx_hbm[:, c16_0:c16_n])
            for (k, cap, slot, c16) in ent:
                table = tables[k]
                for off in range(0, cap, GMAXI):
                    sub = min(GMAXI, cap - off)
                    so = slot + off
                    co = c16 - c16_0 + off // 16
                    ci = call_i[0]
                    call_i[0] += 1
                    nval = cnt_regs[ci % 4]
                    nc.gpsimd.reg_load(nval, cnt_sb[0:1, ci : ci + 1])
                    nc.gpsimd.dma_gather(
                        gbuf[:, so // 128 : (so + sub) // 128, :],
                        table,
                        itile[:, co : co + sub // 16],
                        sub, nval, F,
                        queue_num=qc[0] % NQ,
                    )
                    qc[0] += 1
            return gbuf, tot // 128

        # =============== Phase 1: L1 windows + fused node transform =======
        x_tables = [x_bf[k * CHUNK_X : min((k + 1) * CHUNK_X, cfg["N"]), :]
                    for k in range(cfg["n_chunks_x"])]
        with tc.tile_pool(name="gp1", bufs=3) as gpool, \
             tc.tile_pool(name="ip1", bufs=4) as ipool, \
             tc.tile_pool(name="mp1", bufs=4) as mpool, \
             tc.tile_pool(name="sp1", bufs=8) as spool, \
             tc.tile_pool(name="sb1", bufs=3) as sbp, \
             tc.tile_pool(name="ps_agg", bufs=2, space="PSUM") as psA, \
             tc.tile_pool(name="ps_h1", bufs=2, space="PSUM") as psB, \
             tc.tile_pool(name="ps_z", bufs=2, space="PSUM") as psC:
            pending_cc = []
            call1 = [0]
            for j in range(ngrp):
                for w in range(grp_w0[j], grp_w0[j + 1]):
                    gbuf, nt = gather_window(gpool, ipool, w, sched1, idx1,
                                             x_tables, T1, F1P, "l1", w < 3,
                                             cnt1_sb, call1)
                    meta = mpool.tile([128, 4 * T1], FP32, tag="meta",
                                      name=f"m1_{w}")
                    nc.sync.dma_start(meta[:], meta1[w])
                    xw = mpool.tile([80, DSTW], FP32, tag="xtn",
                                    name=f"xw_{w}")
                    nc.sync.dma_start(
                        xw[:], xtn_in[:, w * DSTW : (w + 1) * DSTW])
                    agg = psA.tile([80, DSTW], FP32, tag="agg",
                                   name=f"agg_{w}")
                    for t in range(nt):
                        S = build_S(spool, spool, w, t, meta, T1, "l1")
                        nc.tensor.matmul(agg[:], gbuf[:, t, 0:80], S[:],
                                         start=(t == 0), stop=(t == nt - 1))
                    asb = sbp.tile([80, DSTW], BF16, tag="asb",
                                   name=f"asb_{w}")
                    nc.vector.tensor_tensor(asb[:], agg[:], xw[:], ALU.add)
                    zp = [psC.tile([128, F2P], FP32, tag=f"zp{h}",
                                   name=f"zp_{w}_{h}") for h in range(2)]
                    for ki in range(3):
                        hp = psB.tile([128, DSTW], FP32, tag="hp",
                                      name=f"hp_{w}_{ki}")
                        nc.tensor.matmul(
                            hp[:], w1_sb[0:79, ki * 128 : (ki + 1) * 128],
                            asb[0:79, :], start=True, stop=True)
                        ht = sbp.tile([128, DSTW], BF16, tag="ht",
                                      name=f"ht_{w}_{ki}")
                        nc.scalar.activation(ht[:], hp[:], AF.Relu)
                        for h in range(2):
                            nc.tensor.matmul(
                                zp[h][:], ht[:, h * 128 : (h + 1) * 128],
                                w2_sb[ki][:], start=(ki == 0), stop=(ki == 2))
                    for h in range(2):
                        zsb = sbp.tile([128, F2P], BF16, tag="zsb",
                                       name=f"zsb_{w}_{h}")
                        nc.scalar.activation(zsb[:], zp[h][:], AF.Copy)
                        nc.sync.dma_start(
                            z_me[w * DSTW + h * 128 : w * DSTW + (h + 1) * 128,
                                 :], zsb[:])
                # lag-one-group collective issue to avoid stalling gathers
                pending_cc.append(j)
                if upto >= 2 and len(pending_cc) > 1:
                    jj = pending_cc.pop(0)
                    nc.gpsimd.collective_compute(
                        "AllGather", ALU.bypass,
                        replica_groups=[list(range(n_cores))],
                        ins=[z_me[grp_r0[jj] : grp_r0[jj + 1], :].opt()],
                        outs=[z_full[jj][:].opt()],
                    )
            for jj in (pending_cc if upto >= 2 else []):
                nc.gpsimd.collective_compute(
                    "AllGather", ALU.bypass,
                    replica_groups=[list(range(n_cores))],
                    ins=[z_me[grp_r0[jj] : grp_r0[jj + 1], :].opt()],
                    outs=[z_full[jj][:].opt()],
                )

        if upto == 1:
            nc.sync.dma_start(dbg1[:], z_me[:])
        if upto == 2:
            nc.sync.dma_start(dbg2[:], z_full[0][:])

        # =============== Phase 2: L2 windows + raw-agg pooling =============
        z_tables = [z_full[k][:] for k in range(ngrp)]
        with tc.tile_pool(name="gp2", bufs=3) as gpool, \
             tc.tile_pool(name="ip2", bufs=4) as ipool, \
             tc.tile_pool(name="mp2", bufs=4) as mpool, \
             tc.tile_pool(name="sp2", bufs=8) as spool, \
             tc.tile_pool(name="ps_a2", bufs=2, space="PSUM") as ps2:
            call2 = [0]
            for w in range(n_win2 if upto >= 4 else 0):
                gbuf, nt = gather_window(gpool, ipool, w, sched2, idx2,
                                         z_tables, T2, F2P, "l2", w < 3,
                                         cnt2_sb, call2)
                meta = mpool.tile([128, 4 * T2], FP32, tag="meta",
                                  name=f"m2_{w}")
                nc.sync.dma_start(meta[:], meta2[w])
                aggs = [ps2.tile([128, DSTW], FP32, tag=f"a2_{fi}",
                                 name=f"a2_{w}_{fi}") for fi in range(3)]
                for t in range(nt):
                    S = build_S(spool, spool, w, t, meta, T2, "l2")
                    for fi in range(3):
                        nc.tensor.matmul(
                            aggs[fi][:], gbuf[:, t, fi * 128 : (fi + 1) * 128],
                            S[:], start=(t == 0), stop=(t == nt - 1))
                gl, sw = w // Wmax, w % Wmax
                for fi in range(3):
                    nc.vector.tensor_reduce(
                        pooled_win[fi][:, gl, sw : sw + 1], aggs[fi][:],
                        axis=mybir.AxisListType.X, op=ALU.max)

        # =============== Phase 3: pool combine + MLP =======================
        if upto >= 4:
            with tc.tile_pool(name="p5", bufs=2) as p5, \
                 tc.tile_pool(name="ps_mlp", bufs=4, space="PSUM") as psz, \
                 tc.tile_pool(name="zsb5", bufs=1) as zsbp:
                pooledTr = []
                for m in range(3):
                    praw = p5.tile([128, G], FP32, tag="praw",
                                   name=f"praw{m}")
                    nc.vector.tensor_reduce(
                        praw[:], pooled_win[m][:],
                        axis=mybir.AxisListType.X, op=ALU.max)
                    pr = zsbp.tile([128, G], BF16, name=f"pTr{m}")
                    nc.scalar.activation(pr[:], praw[:], AF.Relu,
                                         bias=bias_sb[:, m : m + 1])
                    pooledTr.append(pr)
                z1t = []
                for mi in range(8):
                    zp = psz.tile([128, G], FP32, tag="z1p",
                                  name=f"z1p_{mi}")
                    for ki in range(3):
                        nc.tensor.matmul(
                            zp[:],
                            w3_sb[ki][:, mi * 128 : (mi + 1) * 128],
                            pooledTr[ki][:], start=(ki == 0), stop=(ki == 2))
                    zt = zsbp.tile([128, G], BF16, name=f"z1t_{mi}")
                    nc.scalar.activation(zt[:], zp[:], AF.Relu,
                                         bias=bias_sb[:, 3 + mi : 4 + mi])
                    z1t.append(zt)
                zp2 = psz.tile([G, FOUT], FP32, tag="z2p", name="z2p")
                for ki in range(9):
                    lhsT = z1t[ki][:] if ki < 8 else ones64[:]
                    rhs = w4_sb[ki][:] if ki < 8 else b4_sb[:]
                    nc.tensor.matmul(zp2[:], lhsT, rhs,
                                     start=(ki == 0), stop=(ki == 8))
                zfin = zsbp.tile([G, FOUT], FP32, name="zfin")
                nc.scalar.activation(zfin[:], zp2[:], AF.Relu)
                nc.sync.dma_start(z_out[:], zfin[:])

    nc.compile()
    nc.generate_event_semaphores()
    return nc


# ======================= public entry point =======================
_NC_CACHE = {}


def kernel(x, edge_index, batch, W1, b1, W2, b2, W3, b3, W4, b4,
           trace=False, upto=5):
    weights = dict(W1=np.asarray(W1, np.float32), b1=np.asarray(b1, np.float32),
                   W2=np.asarray(W2, np.float32), b2=np.asarray(b2, np.float32),
                   W3=np.asarray(W3, np.float32), b3=np.asarray(b3, np.float32),
                   W4=np.asarray(W4, np.float32), b4=np.asarray(b4, np.float32))
    n_cores = 8
    cfg, per_core, shared = build_plan(
        np.asarray(x, np.float32), np.asarray(edge_index), np.asarray(batch),
        weights, n_graphs=512, n_cores=n_cores)
    key = (upto, cfg["N"], cfg["NMAX"], cfg["n_win1"], cfg["n_win2"],
           cfg["T1"], cfg["T2"], cfg["n_idx16_1"], cfg["n_idx16_2"])
    if key not in _NC_CACHE:
        _NC_CACHE[key] = build_kernel(cfg, n_cores=n_cores, upto=upto)
    nc = _NC_CACHE[key]

    def bf16(a):
        import ml_dtypes
        return np.asarray(a).astype(ml_dtypes.bfloat16)

    base = dict(
        x_bf=bf16(shared["x_bf"]), w1aug=bf16(shared["w1aug"]),
        w2aug=bf16(shared["w2aug"]), w3aug=bf16(shared["w3aug"]),
        w4aug=bf16(shared["w4aug"]), b4row=bf16(shared["b4row"]),
        biases=shared["biases"],
    )
    maps = []
    for pc in per_core:
        m = dict(base)
        m["idx1"] = pc["idx1"]
        m["idx2"] = pc["idx2"]
        m["meta1"] = pc["meta1"]
        m["meta2"] = pc["meta2"]
        m["xtn"] = pc["xtn"]
        m["cnt1"] = pc["cnt1"]
        m["cnt2"] = pc["cnt2"]
        maps.append(m)
    res = run_bass_kernel_spmd(nc, maps, core_ids=list(range(n_cores)),
                               trace=trace)
    z = np.concatenate([res.results[c]["z"] for c in range(n_cores)], axis=0)
    kernel.last_results = res
    return z.astype(np.float32)


# revision 19
# speedup vs baseline: 2.9544x; 1.1732x over previous
"""Trainium2 Bass kernel for nn_DrugGCNncoder (2-layer GCN + max-pool + MLP).

Self-contained: accepts the FULL inputs of reference.setup_inputs(), shards
across 8 NeuronCores internally (dst-node/graph sharding), returns the FULL
[512, 128] output.

v2 design (vs v1 baseline):
 - bf16 gather tables, S-matrices and weights (2x DVE, half gather bytes).
 - W1 + relu + W2 fused per-node into the L1 window epilogue, producing
   z = relu(agg@W1+b1)@W2 directly; AllGather ships z (no dense phase 3,
   no transposes anywhere).
 - L1 self-loops folded into the epilogue via a host-precomputed
   norm_self * x^T tensor (removes them from the gather).
 - Window max-pool on the RAW aggregate; bias+relu applied after pooling
   (exact because relu is monotone; empty graphs correct because b2 == 0).
 - Graph-uniform window slots (Wmax per graph) -> compile-time segment
   reduce, no pooling masks.
 - Index padding with -1 sentinels: the gather ucode skips trailing -1s,
   so padded slots cost zero descriptors on each core.
 - 4 SWDGE queues, gather calls rotate across them.
 - AllGather split into 4 chunks, issued with one-group lag to overlap
   the transfer with the L1 tail.
"""
import sys
for p in ("/opt/trn_rl_repo", "/root/.axon_site/_ro/trn_rl_repo"):
    if p not in sys.path:
        sys.path.insert(0, p)
import numpy as np
import concourse.bass as bass
import concourse.bacc as bacc
import concourse.mybir as mybir
from concourse import tile
from concourse.bass_utils import run_bass_kernel_spmd

FP32 = mybir.dt.float32
BF16 = mybir.dt.bfloat16
I16 = mybir.dt.int16
AF = mybir.ActivationFunctionType
ALU = mybir.AluOpType

CHUNK_X = 32768      # x-table chunk rows (int16 index range)
DSTW = 256           # window width in dst-node columns
F1P = 128            # x padded feature count (bf16 -> 256B rows)
F2P = 384            # z padded feature count (bf16 -> 768B rows)
F1 = 78
F2 = 300
FOUT = 128
N_CORES = 8
N_GRAPHS = 512
GMAXI = 512          # max rows per dma_gather call
NQ = 4               # SWDGE queues


def _pack_idx16(idx, cap):
    """idx (valid list) -> [128, cap//16] int16, slot j at [j%16, j//16],
    padded with -1 (skipped by the gather ucode), replicated 8x."""
    assert cap % 16 == 0 and len(idx) <= cap
    full = np.full(cap, -1, np.int16)
    full[: len(idx)] = idx
    blk = full.reshape(cap // 16, 16).T  # [16, cap/16]
    return np.tile(blk, (8, 1))  # [128, cap/16]


def build_plan(x, edge_index, batch, weights, n_graphs=512, n_cores=8):
    N = x.shape[0]
    G = n_graphs // n_cores
    src = edge_index[0].astype(np.int64)
    dst = edge_index[1].astype(np.int64)
    deg = (np.bincount(dst, minlength=N) + 1).astype(np.float64)  # + self loop
    dis = 1.0 / np.sqrt(deg)
    norm_e = (dis[src] * dis[dst]).astype(np.float32)
    norm_self = (dis * dis).astype(np.float32)

    batch = batch.astype(np.int64)
    g_start = np.searchsorted(batch, np.arange(n_graphs), side="left")
    g_end = np.searchsorted(batch, np.arange(n_graphs), side="right")
    node_start = [int(g_start[c * G]) for c in range(n_cores)]
    node_start.append(N)
    nodes_per_core = [node_start[c + 1] - node_start[c] for c in range(n_cores)]
    NMAX = ((max(nodes_per_core) + DSTW - 1) // DSTW) * DSTW
    n_win1 = NMAX // DSTW

    core_of = np.searchsorted(np.asarray(node_start[1:]), np.arange(N),
                              side="right")
    local_of = np.arange(N) - np.asarray(node_start)[core_of]

    # ---- z-table chunking (for chunked AllGather + int16 range) ----------
    ngrp = 4
    base_w = n_win1 // ngrp
    extra = n_win1 - base_w * ngrp
    grp_sizes = [base_w + (1 if j < extra else 0) for j in range(ngrp)]
    grp_w0 = np.cumsum([0] + grp_sizes)          # window offsets, len 5
    grp_rows = [s * DSTW for s in grp_sizes]     # local rows per group
    grp_r0 = np.cumsum([0] + grp_rows)           # local row offsets, len 5
    assert all(8 * r <= 32768 for r in grp_rows)
    # map local row -> (group j, row within z_full[j]) for a given core
    grp_of_local = np.searchsorted(grp_r0[1:], np.arange(NMAX), side="right")

    # ---- per-core dst-sorted edges --------------------------------------
    per_core_raw = []
    for c in range(n_cores):
        sel = (dst >= node_start[c]) & (dst < node_start[c + 1])
        s, d, nm = src[sel], dst[sel], norm_e[sel]
        dl = d - node_start[c]
        order = np.argsort(dl, kind="stable")
        per_core_raw.append((s[order], dl[order], nm[order]))

    # ---- L2 graph-uniform windows ---------------------------------------
    g_len = (g_end - g_start).astype(np.int64)
    Wmax = max(1, int((g_len.max() + DSTW - 1) // DSTW))
    n_win2 = G * Wmax

    # L1: source row in x table; chunk by global id // 32768
    n_chunks_x = (N + CHUNK_X - 1) // CHUNK_X
    # L2: source row in z_full[j]; j from the SOURCE node's local offset
    src_grp = grp_of_local[np.minimum(local_of, NMAX - 1)]
    src_zrow = (core_of * np.asarray(grp_rows)[src_grp]
                + (local_of - np.asarray(grp_r0)[src_grp]))
    n_chunks_z = ngrp

    def windows_l1(c):
        s_loc, dl, nm = per_core_raw[c]
        out = []
        for w in range(n_win1):
            lo = np.searchsorted(dl, w * DSTW, side="left")
            hi = np.searchsorted(dl, (w + 1) * DSTW, side="left")
            es, edl, enm = s_loc[lo:hi], dl[lo:hi] - w * DSTW, nm[lo:hi]
            ch = es // CHUNK_X
            runs = []
            for k in range(n_chunks_x):
                m = ch == k
                runs.append((es[m] - k * CHUNK_X, edl[m], enm[m]))
            out.append(runs)
        return out

    def windows_l2(c):
        s_loc, dl, nm = per_core_raw[c]
        # self-loop edges for this core's own nodes (kept in the L2 gather)
        own = np.arange(node_start[c], node_start[c + 1])
        sl_dl = own - node_start[c]
        all_src = np.concatenate([s_loc, own])
        all_dl = np.concatenate([dl, sl_dl])
        all_nm = np.concatenate([nm, norm_self[own]]).astype(np.float32)
        order = np.argsort(all_dl, kind="stable")
        all_src, all_dl, all_nm = all_src[order], all_dl[order], all_nm[order]
        out = []
        for gl in range(G):
            g = c * G + gl
            glo = int(g_start[g] - node_start[c])
            ghi = int(g_end[g] - node_start[c])
            for swin in range(Wmax):
                base = glo + swin * DSTW
                top = min(base + DSTW, ghi)
                lo = np.searchsorted(all_dl, base, side="left")
                hi = np.searchsorted(all_dl, max(top, base), side="left")
                es = all_src[lo:hi]
                edl = all_dl[lo:hi] - base
                enm = all_nm[lo:hi]
                rows = src_zrow[es]
                ch = src_grp[es] if len(es) else np.zeros(0, np.int64)
                runs = []
                for k in range(n_chunks_z):
                    m = ch == k
                    runs.append((rows[m], edl[m], enm[m]))
                out.append(runs)
        return out

    l1_cores = [windows_l1(c) for c in range(n_cores)]
    l2_cores = [windows_l2(c) for c in range(n_cores)]

    def normalize(cores_wins, n_win, n_chunks, force_first=False):
        caps = np.zeros((n_win, n_chunks), np.int64)
        for wins in cores_wins:
            for w in range(n_win):
                for k in range(n_chunks):
                    caps[w, k] = max(caps[w, k], len(wins[w][k][0]))
        caps = ((caps + 127) // 128) * 128
        if force_first:
            caps[:, 0] = np.maximum(caps[:, 0], 128)
        T = int(caps.sum(axis=1).max()) // 128
        return caps, T

    caps1, T1 = normalize(l1_cores, n_win1, n_chunks_x, force_first=True)
    caps2, T2 = normalize(l2_cores, n_win2, n_chunks_z, force_first=True)

    def emit(cores_wins, caps, n_win, T, n_chunks):
        n_idx16 = int(caps.sum()) // 16
        out = []
        for wins in cores_wins:
            idx16 = np.full((128, n_idx16), -1, np.int16)
            meta = np.zeros((n_win, 128, 4 * T), np.float32)
            meta[:, :, :T] = -1.0       # dstl pad
            meta[:, :, 2 * T : 3 * T] = 1.0  # -dstl pad
            col16 = 0
            for w in range(n_win):
                slot = 0
                for k in range(n_chunks):
                    cap = int(caps[w, k])
                    ri, rd, rn = wins[w][k]
                    idx16[:, col16 : col16 + cap // 16] = _pack_idx16(ri, cap)
                    n = len(ri)
                    sl = slot + np.arange(n)
                    rdf = rd.astype(np.float32)
                    meta[w, sl % 128, sl // 128] = rdf
                    meta[w, sl % 128, T + sl // 128] = rn
                    meta[w, sl % 128, 2 * T + sl // 128] = -rdf
                    meta[w, sl % 128, 3 * T + sl // 128] = -rn
                    slot += cap
                    col16 += cap // 16
                assert slot <= T * 128
            out.append({"idx16": idx16,
                        "meta": meta.astype(np.float32)})  # cast to bf16 later
        return out

    l1_data = emit(l1_cores, caps1, n_win1, T1, n_chunks_x)
    l2_data = emit(l2_cores, caps2, n_win2, T2, n_chunks_z)

    def call_counts(cores_wins, caps, n_win, n_chunks):
        """Per-core valid-index count for every dma_gather call, in issue
        order (w, then k with cap>0, then GMAXI sub-calls)."""
        out = []
        for wins in cores_wins:
            cnts = []
            for w in range(n_win):
                for k in range(n_chunks):
                    cap = int(caps[w, k])
                    if cap == 0:
                        continue
                    nvalid = len(wins[w][k][0])
                    for off in range(0, cap, GMAXI):
                        sub = min(GMAXI, cap - off)
                        cnts.append(max(0, min(sub, nvalid - off)))
            out.append(np.asarray(cnts, np.int32).reshape(1, -1))
        return out

    cnt1 = call_counts(l1_cores, caps1, n_win1, n_chunks_x)
    cnt2 = call_counts(l2_cores, caps2, n_win2, n_chunks_z)

    def sched(caps):
        rows = []
        col16 = 0
        for w in range(caps.shape[0]):
            slot = 0
            ent = []
            for k in range(caps.shape[1]):
                cap = int(caps[w, k])
                if cap > 0:
                    ent.append((k, cap, slot, col16))
                slot += cap
                col16 += cap // 16
            rows.append((ent, slot))
        return rows

    # ---- packed weights (bf16) ------------------------------------------
    W1, b1, W2, b2, W3, b3, W4, b4 = (
        weights["W1"], weights["b1"], weights["W2"], weights["b2"],
        weights["W3"], weights["b3"], weights["W4"], weights["b4"],
    )
    w1aug = np.zeros((80, F2P), np.float32)
    w1aug[:F1, :F2] = W1
    w1aug[F1, :F2] = b1       # ones-row slot 78
    w2aug = np.zeros((F2P, F2P), np.float32)
    w2aug[:F2, :F2] = W2
    w3aug = np.zeros((F2P, 1024), np.float32)
    w3aug[:F2, :] = W3
    w4aug = np.zeros((1024, FOUT), np.float32)
    w4aug[:, :] = W4
    b4row = b4.reshape(1, FOUT).astype(np.float32)
    biases = np.zeros((128, 11), np.float32)
    for m in range(3):
        seg = np.zeros(128, np.float32)
        seg[: max(0, min(128, F2 - m * 128))] = b2[m * 128 : (m + 1) * 128]
        biases[:, m] = seg
    for m in range(8):
        biases[:, 3 + m] = b3[m * 128 : (m + 1) * 128]

    # x table bf16 [N, 128]
    x_bf = np.zeros((N, F1P), np.float32)
    x_bf[:, :F1] = x

    # per-core norm_self * x^T with ones row at 78
    xtn = []
    for c in range(n_cores):
        t = np.zeros((80, NMAX), np.float32)
        nn = nodes_per_core[c]
        own = np.arange(node_start[c], node_start[c + 1])
        t[:F1, :nn] = (x[own] * norm_self[own][:, None]).T
        t[F1, :] = 1.0
        xtn.append(t)

    cfg = dict(
        N=N, G=G, NMAX=NMAX, n_win1=n_win1, n_win2=n_win2, Wmax=Wmax,
        T1=T1, T2=T2, n_chunks_x=n_chunks_x, n_chunks_z=n_chunks_z,
        sched1=sched(caps1), sched2=sched(caps2),
        n_idx16_1=int(caps1.sum()) // 16, n_idx16_2=int(caps2.sum()) // 16,
        grp_sizes=grp_sizes, grp_rows=grp_rows,
        grp_w0=[int(v) for v in grp_w0], grp_r0=[int(v) for v in grp_r0],
        n_cores=n_cores, n_calls1=cnt1[0].shape[1], n_calls2=cnt2[0].shape[1],
    )
    shared = dict(x_bf=x_bf, w1aug=w1aug, w2aug=w2aug, w3aug=w3aug,
                  w4aug=w4aug, b4row=b4row, biases=biases)
    per_core = []
    for c in range(n_cores):
        per_core.append(dict(
            idx1=l1_data[c]["idx16"], meta1=l1_data[c]["meta"],
            idx2=l2_data[c]["idx16"], meta2=l2_data[c]["meta"],
            xtn=xtn[c], cnt1=cnt1[c], cnt2=cnt2[c],
        ))
    return cfg, per_core, shared


def build_kernel(cfg, n_cores=8, upto=5):
    G = cfg["G"]
    NMAX, n_win1, n_win2 = cfg["NMAX"], cfg["n_win1"], cfg["n_win2"]
    Wmax = cfg["Wmax"]
    T1, T2 = cfg["T1"], cfg["T2"]
    sched1, sched2 = cfg["sched1"], cfg["sched2"]
    grp_rows, grp_w0, grp_r0 = cfg["grp_rows"], cfg["grp_w0"], cfg["grp_r0"]
    ngrp = len(grp_rows)

    nc = bacc.Bacc("TRN2", target_bir_lowering=False, debug=False,
                   num_devices=n_cores, num_swdge_queues=NQ)

    # ---- I/O ----
    x_bf = nc.dram_tensor("x_bf", [cfg["N"], F1P], BF16, kind="ExternalInput")
    xtn_in = nc.dram_tensor("xtn", [80, NMAX], FP32, kind="ExternalInput")
    idx1 = nc.dram_tensor("idx1", [128, cfg["n_idx16_1"]], I16,
                          kind="ExternalInput")
    idx2 = nc.dram_tensor("idx2", [128, cfg["n_idx16_2"]], I16,
                          kind="ExternalInput")
    meta1 = nc.dram_tensor("meta1", [n_win1, 128, 4 * T1], FP32,
                           kind="ExternalInput")
    meta2 = nc.dram_tensor("meta2", [n_win2, 128, 4 * T2], FP32,
                           kind="ExternalInput")
    w1_in = nc.dram_tensor("w1aug", [80, F2P], BF16, kind="ExternalInput")
    w2_in = nc.dram_tensor("w2aug", [F2P, F2P], BF16, kind="ExternalInput")
    w3_in = nc.dram_tensor("w3aug", [F2P, 1024], BF16, kind="ExternalInput")
    w4_in = nc.dram_tensor("w4aug", [1024, FOUT], BF16, kind="ExternalInput")
    b4_in = nc.dram_tensor("b4row", [1, FOUT], BF16, kind="ExternalInput")
    bias_in = nc.dram_tensor("biases", [128, 11], FP32, kind="ExternalInput")
    cnt1_in = nc.dram_tensor("cnt1", [1, cfg["n_calls1"]], mybir.dt.int32,
                             kind="ExternalInput")
    cnt2_in = nc.dram_tensor("cnt2", [1, cfg["n_calls2"]], mybir.dt.int32,
                             kind="ExternalInput")
    z_out = nc.dram_tensor("z", [G, FOUT], FP32, kind="ExternalOutput")
    if upto == 1:
        dbg1 = nc.dram_tensor("dbg1", [NMAX, F2P], BF16, kind="ExternalOutput")
    if upto == 2:
        dbg2 = nc.dram_tensor("dbg2", [8 * grp_rows[0], F2P], BF16,
                              kind="ExternalOutput")

    with tile.TileContext(nc) as tc, \
         tc.tile_pool(name="dram", bufs=1, space="DRAM") as drp, \
         tc.tile_pool(name="consts", bufs=1) as consts:
        z_me = drp.tile([NMAX, F2P], BF16, name="z_me")
        z_full = [drp.tile([n_cores * grp_rows[j], F2P], BF16,
                           addr_space="Shared", name=f"z_full{j}")
                  for j in range(ngrp)]

        iota_i32 = consts.tile([128, DSTW], mybir.dt.int32)
        nc.gpsimd.iota(iota_i32[:], [[1, DSTW]], base=0, channel_multiplier=0)
        iota_bf = consts.tile([128, DSTW], BF16)
        nc.vector.tensor_copy(iota_bf[:], iota_i32[:])
        w1_sb = consts.tile([80, F2P], BF16)
        nc.sync.dma_start(w1_sb[:], w1_in[:])
        w2_sb = [consts.tile([128, F2P], BF16, name=f"w2_{k}") for k in range(3)]
        for k in range(3):
            nc.sync.dma_start(w2_sb[k][:], w2_in[k * 128 : (k + 1) * 128, :])
        w3_sb = [consts.tile([128, 1024], BF16, name=f"w3_{k}") for k in range(3)]
        for k in range(3):
            nc.sync.dma_start(w3_sb[k][:], w3_in[k * 128 : (k + 1) * 128, :])
        w4_sb = [consts.tile([128, FOUT], BF16, name=f"w4_{k}") for k in range(8)]
        for k in range(8):
            nc.sync.dma_start(w4_sb[k][:], w4_in[k * 128 : (k + 1) * 128, :])
        b4_sb = consts.tile([1, FOUT], BF16)
        nc.sync.dma_start(b4_sb[:], b4_in[:])
        bias_sb = consts.tile([128, 11], FP32)
        nc.sync.dma_start(bias_sb[:], bias_in[:])
        ones64 = consts.tile([1, G], BF16)
        nc.vector.memset(ones64[:], 1.0)
        cnt1_sb = consts.tile([1, cfg["n_calls1"]], mybir.dt.int32)
        nc.sync.dma_start(cnt1_sb[:], cnt1_in[:])
        cnt2_sb = consts.tile([1, cfg["n_calls2"]], mybir.dt.int32)
        nc.sync.dma_start(cnt2_sb[:], cnt2_in[:])
        cnt_regs = [nc.gpsimd.alloc_register(f"cnt_reg{i}") for i in range(4)]
        pooled_win = [consts.tile([128, G, Wmax], FP32, name=f"pw{m}")
                      for m in range(3)]

        qc = [0]

        s_ctr = [0]

        def build_S(spool, tpool, w, t, meta, T, tag):
            """S[p, j] = norm_p * 1[iota_j == dstl_p], routed to DVE or ACT."""
            S = spool.tile([128, DSTW], BF16, tag="S", name=f"S_{tag}_{w}_{t}")
            i = s_ctr[0]
            s_ctr[0] += 1
            if i % 5 < 4:
                nc.vector.tensor_scalar(
                    S[:], iota_bf[:], meta[:, t : t + 1],
                    meta[:, T + t : T + t + 1], ALU.is_equal, ALU.mult)
            else:
                sq = tpool.tile([128, DSTW], BF16, tag="sq",
                                name=f"sq_{tag}_{w}_{t}")
                nc.scalar.activation(sq[:], iota_bf[:], AF.Square,
                                     bias=meta[:, 2 * T + t : 2 * T + t + 1])
                nc.scalar.activation(S[:], sq[:], AF.Relu,
                                     scale=meta[:, 3 * T + t : 3 * T + t + 1],
                                     bias=meta[:, T + t : T + t + 1])
            return S

        def gather_window(gpool, ipool, w, sched, idx_hbm, tables, T, F, tag,
                          memset_first, cnt_sb, call_i):
            ent, tot = sched[w]
            gbuf = gpool.tile([128, T, F], BF16, tag="gbuf",
                              name=f"gbuf_{tag}_{w}", padded_shape=[128, T, F])
            if memset_first:
                nc.vector.memset(gbuf[:], 0.0)
            c16_0 = ent[0][3]
            c16_n = ent[-1][3] + ent[-1][1] // 16
            itile = ipool.tile([128, c16_n - c16_0], I16, tag="idx",
                               name=f"idx_{tag}_{w}")
            nc.sync.dma_start(itile[:], idx_hbm[:, c16_0:c16_n])
            for (k, cap, slot, c16) in ent:
                table = tables[k]
                for off in range(0, cap, GMAXI):
                    sub = min(GMAXI, cap - off)
                    so = slot + off
                    co = c16 - c16_0 + off // 16
                    ci = call_i[0]
                    call_i[0] += 1
                    nval = cnt_regs[ci % 4]
                    nc.gpsimd.reg_load(nval, cnt_sb[0:1, ci : ci + 1])
                    nc.gpsimd.dma_gather(
                        gbuf[:, so // 128 : (so + sub) // 128, :],
                        table,
                        itile[:, co : co + sub // 16],
                        sub, nval, F,
                        queue_num=qc[0] % NQ,
                    )
                    qc[0] += 1
            return gbuf, tot // 128

        # =============== Phase 1: L1 windows + fused node transform =======
        x_tables = [x_bf[k * CHUNK_X : min((k + 1) * CHUNK_X, cfg["N"]), :]
                    for k in range(cfg["n_chunks_x"])]
        with tc.tile_pool(name="gp1", bufs=3) as gpool, \
             tc.tile_pool(name="ip1", bufs=4) as ipool, \
             tc.tile_pool(name="mp1", bufs=4) as mpool, \
             tc.tile_pool(name="sp1", bufs=24) as spool, \
             tc.tile_pool(name="sb1", bufs=3) as sbp, \
             tc.tile_pool(name="ps_agg", bufs=2, space="PSUM") as psA, \
             tc.tile_pool(name="ps_h1", bufs=2, space="PSUM") as psB, \
             tc.tile_pool(name="ps_z", bufs=2, space="PSUM") as psC:
            pending_cc = []
            call1 = [0]
            for j in range(ngrp):
                for w in range(grp_w0[j], grp_w0[j + 1]):
                    gbuf, nt = gather_window(gpool, ipool, w, sched1, idx1,
                                             x_tables, T1, F1P, "l1", w < 3,
                                             cnt1_sb, call1)
                    meta = mpool.tile([128, 4 * T1], FP32, tag="meta",
                                      name=f"m1_{w}")
                    nc.sync.dma_start(meta[:], meta1[w])
                    xw = mpool.tile([80, DSTW], FP32, tag="xtn",
                                    name=f"xw_{w}")
                    nc.sync.dma_start(
                        xw[:], xtn_in[:, w * DSTW : (w + 1) * DSTW])
                    agg = psA.tile([80, DSTW], FP32, tag="agg",
                                   name=f"agg_{w}")
                    for t in range(nt):
                        S = build_S(spool, spool, w, t, meta, T1, "l1")
                        nc.tensor.matmul(agg[:], gbuf[:, t, 0:80], S[:],
                                         start=(t == 0), stop=(t == nt - 1))
                    asb = sbp.tile([80, DSTW], BF16, tag="asb",
                                   name=f"asb_{w}")
                    nc.vector.tensor_tensor(asb[:], agg[:], xw[:], ALU.add)
                    zp = [psC.tile([128, F2P], FP32, tag=f"zp{h}",
                                   name=f"zp_{w}_{h}") for h in range(2)]
                    for ki in range(3):
                        hp = psB.tile([128, DSTW], FP32, tag="hp",
                                      name=f"hp_{w}_{ki}")
                        nc.tensor.matmul(
                            hp[:], w1_sb[0:79, ki * 128 : (ki + 1) * 128],
                            asb[0:79, :], start=True, stop=True)
                        ht = sbp.tile([128, DSTW], BF16, tag="ht",
                                      name=f"ht_{w}_{ki}")
                        nc.scalar.activation(ht[:], hp[:], AF.Relu)
                        for h in range(2):
                            nc.tensor.matmul(
                                zp[h][:], ht[:, h * 128 : (h + 1) * 128],
                                w2_sb[ki][:], start=(ki == 0), stop=(ki == 2))
                    for h in range(2):
                        zsb = sbp.tile([128, F2P], BF16, tag="zsb",
                                       name=f"zsb_{w}_{h}")
                        nc.scalar.activation(zsb[:], zp[h][:], AF.Copy)
                        nc.sync.dma_start(
                            z_me[w * DSTW + h * 128 : w * DSTW + (h + 1) * 128,
                                 :], zsb[:])
                # lag-one-group collective issue to avoid stalling gathers
                pending_cc.append(j)
                if upto >= 2 and len(pending_cc) > 1:
                    jj = pending_cc.pop(0)
                    nc.gpsimd.collective_compute(
                        "AllGather", ALU.bypass,
                        replica_groups=[list(range(n_cores))],
                        ins=[z_me[grp_r0[jj] : grp_r0[jj + 1], :].opt()],
                        outs=[z_full[jj][:].opt()],
                    )
            for jj in (pending_cc if upto >= 2 else []):
                nc.gpsimd.collective_compute(
                    "AllGather", ALU.bypass,
                    replica_groups=[list(range(n_cores))],
                    ins=[z_me[grp_r0[jj] : grp_r0[jj + 1], :].opt()],
                    outs=[z_full[jj][:].opt()],
                )

        if upto == 1:
            nc.sync.dma_start(dbg1[:], z_me[:])
        if upto == 2:
            nc.sync.dma_start(dbg2[:], z_full[0][:])

        # =============== Phase 2: L2 windows + raw-agg pooling =============
        z_tables = [z_full[k][:] for k in range(ngrp)]
        with tc.tile_pool(name="gp2", bufs=3) as gpool, \
             tc.tile_pool(name="ip2", bufs=4) as ipool, \
             tc.tile_pool(name="mp2", bufs=4) as mpool, \
             tc.tile_pool(name="sp2", bufs=24) as spool, \
             tc.tile_pool(name="ps_a2", bufs=2, space="PSUM") as ps2:
            call2 = [0]
            for w in range(n_win2 if upto >= 4 else 0):
                gbuf, nt = gather_window(gpool, ipool, w, sched2, idx2,
                                         z_tables, T2, F2P, "l2", w < 3,
                                         cnt2_sb, call2)
                meta = mpool.tile([128, 4 * T2], FP32, tag="meta",
                                  name=f"m2_{w}")
                nc.sync.dma_start(meta[:], meta2[w])
                aggs = [ps2.tile([128, DSTW], FP32, tag=f"a2_{fi}",
                                 name=f"a2_{w}_{fi}") for fi in range(3)]
                for t in range(nt):
                    S = build_S(spool, spool, w, t, meta, T2, "l2")
                    for fi in range(3):
                        nc.tensor.matmul(
                            aggs[fi][:], gbuf[:, t, fi * 128 : (fi + 1) * 128],
                            S[:], start=(t == 0), stop=(t == nt - 1))
                gl, sw = w // Wmax, w % Wmax
                for fi in range(3):
                    nc.vector.tensor_reduce(
                        pooled_win[fi][:, gl, sw : sw + 1], aggs[fi][:],
                        axis=mybir.AxisListType.X, op=ALU.max)

        # =============== Phase 3: pool combine + MLP =======================
        if upto >= 4:
            with tc.tile_pool(name="p5", bufs=2) as p5, \
                 tc.tile_pool(name="ps_mlp", bufs=4, space="PSUM") as psz, \
                 tc.tile_pool(name="zsb5", bufs=1) as zsbp:
                pooledTr = []
                for m in range(3):
                    praw = p5.tile([128, G], FP32, tag="praw",
                                   name=f"praw{m}")
                    nc.vector.tensor_reduce(
                        praw[:], pooled_win[m][:],
                        axis=mybir.AxisListType.X, op=ALU.max)
                    pr = zsbp.tile([128, G], BF16, name=f"pTr{m}")
                    nc.scalar.activation(pr[:], praw[:], AF.Relu,
                                         bias=bias_sb[:, m : m + 1])
                    pooledTr.append(pr)
                z1t = []
                for mi in range(8):
                    zp = psz.tile([128, G], FP32, tag="z1p",
                                  name=f"z1p_{mi}")
                    for ki in range(3):
                        nc.tensor.matmul(
                            zp[:],
                            w3_sb[ki][:, mi * 128 : (mi + 1) * 128],
                            pooledTr[ki][:], start=(ki == 0), stop=(ki == 2))
                    zt = zsbp.tile([128, G], BF16, name=f"z1t_{mi}")
                    nc.scalar.activation(zt[:], zp[:], AF.Relu,
                                         bias=bias_sb[:, 3 + mi : 4 + mi])
                    z1t.append(zt)
                zp2 = psz.tile([G, FOUT], FP32, tag="z2p", name="z2p")
                for ki in range(9):
                    lhsT = z1t[ki][:] if ki < 8 else ones64[:]
                    rhs = w4_sb[ki][:] if ki < 8 else b4_sb[:]
                    nc.tensor.matmul(zp2[:], lhsT, rhs,
                                     start=(ki == 0), stop=(ki == 8))
                zfin = zsbp.tile([G, FOUT], FP32, name="zfin")
                nc.scalar.activation(zfin[:], zp2[:], AF.Relu)
                nc.sync.dma_start(z_out[:], zfin[:])

    nc.compile()
    nc.generate_event_semaphores()
    return nc


# ======================= public entry point =======================
_NC_CACHE = {}


def kernel(x, edge_index, batch, W1, b1, W2, b2, W3, b3, W4, b4,
           trace=False, upto=5):
    weights = dict(W1=np.asarray(W1, np.float32), b1=np.asarray(b1, np.float32),
                   W2=np.asarray(W2, np.float32), b2=np.asarray(b2, np.float32),
                   W3=np.asarray(W3, np.float32), b3=np.asarray(b3, np.float32),
                   W4=np.asarray(W4, np.float32), b4=np.asarray(b4, np.float32))
    n_cores = 8
    cfg, per_core, shared = build_plan(
        np.asarray(x, np.float32), np.asarray(edge_index), np.asarray(batch),
        weights, n_graphs=512, n_cores=n_cores)
    key = (upto, cfg["N"], cfg["NMAX"], cfg["n_win1"], cfg["n_win2"],
           cfg["T1"], cfg["T2"], cfg["n_idx16_1"], cfg["n_idx16_2"])
    if key not in _NC_CACHE:
        _NC_CACHE[key] = build_kernel(cfg, n_cores=n_cores, upto=upto)
    nc = _NC_CACHE[key]

    def bf16(a):
        import ml_dtypes
        return np.asarray(a).astype(ml_dtypes.bfloat16)

    base = dict(
        x_bf=bf16(shared["x_bf"]), w1aug=bf16(shared["w1aug"]),
        w2aug=bf16(shared["w2aug"]), w3aug=bf16(shared["w3aug"]),
        w4aug=bf16(shared["w4aug"]), b4row=bf16(shared["b4row"]),
        biases=shared["biases"],
    )
    maps = []
    for pc in per_core:
        m = dict(base)
        m["idx1"] = pc["idx1"]
        m["idx2"] = pc["idx2"]
        m["meta1"] = pc["meta1"]
        m["meta2"] = pc["meta2"]
        m["xtn"] = pc["xtn"]
        m["cnt1"] = pc["cnt1"]
        m["cnt2"] = pc["cnt2"]
        maps.append(m)
    res = run_bass_kernel_spmd(nc, maps, core_ids=list(range(n_cores)),
                               trace=trace)
    z = np.concatenate([res.results[c]["z"] for c in range(n_cores)], axis=0)
    kernel.last_results = res
    return z.astype(np.float32)
